# revision 9
# baseline (speedup 1.0000x reference)
"""KAN embeddings Bass kernel for Trainium2, 8-core data-parallel over batch.

out[b,i,d] = silu(x[b,i]) * base_w[i,d] + sum_g exp(-0.5(x[b,i]-grid[g])^2) * gp_w[i,g,d]

v2 strategy: the 64 grid Gaussians (sigma=1, spacing 0.063) are massively
oversampled -- the gp branch is re-expressed in a coarse basis of M=12
sigma-1 Gaussians with centers spanning [grid_min-0.5, grid_max+0.5].
The projection T (ridge least-squares fit on the empirical x distribution,
done per-call on host) folds into the weights: c[m,i,d] = sum_g T[m,g]
gp_w[i,g,d]. Validated offline: rel err 0.0025 vs 0.0027 for the exact-
basis bf16 path, because |c| stays O(1). Weight DMA drops 17 MB -> 3.4 MB
per core; the kernel becomes output-store-bound (~67 MB bf16 out).

Per core (batch shard BL=256), per tile of 4 features:
  - 64-row strip layout (PE operand base partitions must be 0/32/64, so
    two strips): fb4 tile (128, 512); strip u (u=0,1) rows 64u..64u+11 =
    coarse RBF feats, row 64u+12 = silu; cols = feature (f+u)'s 256 batch
    ++ feature (f+u+2)'s 256 batch.
  - RBF args via K=5 broadcast matmuls (double-bf16 split of -x^2/2 and x)
    with 2 zero-padded stationaries (5,128) accumulating into ONE (128,512)
    PSUM bank; ONE exp ACT pass per tile (bias -h^2/2 per partition, -30 on
    pad rows so exp writes ~0 there). ACT cost is free-size-bound, so
    stacking strips halves the exp instruction count vs (64,512) tiles.
  - silu rows DMA'd from DRAM on the ACT HWDGE ring right after the exp
    (same-engine program order gives the WAW ordering for free).
  - main matmuls: K=13 contraction, stationary = fb4 strip (13,128),
    moving = weights (13,512) in a (128, 4D) tile whose 64-row bands match
    the strip base partitions; PE cost is N-bound so small K is free.
    Nothing ever reads the pad rows -> no memsets anywhere.
  - PSUM -> SBUF copies (only ACT and DVE read PSUM) drain 2-bank
    128x1024 tiles, downcast to bf16, balanced so ACT(exp+copies) ~=
    DVE(copies); output DMAs are bf16, issued per (tile, batch-half);
    c=0 stores on the sync HWDGE ring, c=1 stores on gpsimd SWDGE
    (latency-tolerant). Host upcasts bf16 -> f32.
"""

import numpy as np

B, NF, G, D = 2048, 256, 64, 512
NCORES = 8
BL = B // NCORES          # 256 batch rows per core
M = 12                    # coarse Gaussian basis size
KW = M + 1                # weight rows: M coarse + base row
NT = NF // 4              # 64 tiles of 4 features
KF = 5                    # feature-matmul contraction rows

# ACT copies out of 256 total so ACT(exp+copies) ~ DVE(copies)
_NACT = 106

_cache = {}


def _build():
    import concourse.bass as bass
    from concourse import mybir
    from concourse import tile

    f32 = mybir.dt.float32
    bf16 = mybir.dt.bfloat16
    AF = mybir.ActivationFunctionType

    nc = bass.Bass()
    x6 = nc.declare_dram_parameter("x6", [KF + 1, NF * BL], bf16, isOutput=False)
    wcatT = nc.declare_dram_parameter("wcatT", [KW, NF * D], bf16, isOutput=False)
    s5q = nc.declare_dram_parameter("s5q", [KF, 256], bf16, isOutput=False)
    biasv = nc.declare_dram_parameter("biasv", [128, 1], f32, isOutput=False)
    out = nc.declare_dram_parameter("out", [BL, NF, D], bf16, isOutput=True)

    with tile.TileContext(nc) as tc:
        with (
            tc.tile_pool(name="const", bufs=1) as constp,
            tc.tile_pool(name="x6p", bufs=3) as x6p,
            tc.tile_pool(name="fb4p", bufs=3) as fb4p,
            tc.tile_pool(name="wtp", bufs=4) as wtp,
            tc.tile_pool(name="stage", bufs=6) as stagep,
            tc.tile_pool(name="pt", bufs=2, space="PSUM") as ptp,
            tc.tile_pool(name="po", bufs=3, space="PSUM") as pop,
        ):
            s5q_t = constp.tile([KF, 256], bf16)
            nc.sync.dma_start(out=s5q_t[:, :], in_=s5q[:, :])
            biasv_t = constp.tile([128, 1], f32)
            nc.sync.dma_start(out=biasv_t[:, :], in_=biasv[:, :])

            # x6 loads cover 4 tiles each (5, 4096); the silu row (row 5)
            # goes straight DRAM -> fb4 and is never staged
            def issue_x6_load(q):
                if q >= NT // 4:
                    return None
                x6_t = x6p.tile([KF, 4096], bf16)
                nc.sync.dma_start(
                    out=x6_t[:, :], in_=x6[0:KF, q * 4096:(q + 1) * 4096]
                )
                return x6_t

            # weight tile (128, 4D) covers 2 tiles: band u rows 64u..64u+KW-1
            # hold strip u's features for both tiles (matmul needs lhsT/rhs
            # at the same base partition); wcatT is host-permuted so each
            # band DMA reads a contiguous 4D-column span
            def issue_wt_load(g):
                if g >= NT // 2:
                    return None
                wt = wtp.tile([128, 4 * D], bf16)
                for u in range(2):
                    nc.sync.dma_start(
                        out=wt[64 * u:64 * u + KW, :],
                        in_=wcatT[0:KW, (g * 8 + 4 * u) * D:(g * 8 + 4 * u + 4) * D],
                    )
                return wt

            # feature stage for tile t: 2 accumulating matmuls -> 1 exp ->
            # silu-row DMA (ACT ring, ordered after the exp by program order)
            def feature_stage(t, x6_tiles):
                if t >= NT:
                    return None
                x6_t = x6_tiles[t // 4]
                off = (t % 4) * 1024
                pt = ptp.tile([128, 512], f32)
                for u in range(2):
                    nc.tensor.matmul(
                        pt[:, :],
                        s5q_t[0:KF, u * 128:(u + 1) * 128],
                        x6_t[0:KF, off + u * 512:off + (u + 1) * 512],
                        start=(u == 0),
                        stop=(u == 1),
                    )
                fb4 = fb4p.tile([128, 512], bf16)
                nc.scalar.activation(
                    fb4[:, :], pt[:, :], AF.Exp, bias=biasv_t[:, :], scale=1.0
                )
                silu_dst = fb4[:, :].rearrange("(u p) c -> u p c", u=2)[:, 12, :]
                silu_src = x6[KF:KF + 1, t * 1024:(t + 1) * 1024].rearrange(
                    "one (u c) -> (one u) c", u=2
                )
                nc.scalar.dma_start(out=silu_dst, in_=silu_src)
                return fb4

            x6_tiles = {0: issue_x6_load(0), 1: issue_x6_load(1)}
            wts = {0: issue_wt_load(0), 1: issue_wt_load(1)}
            fb4s = {0: feature_stage(0, x6_tiles)}

            ncopy = 0
            nact_done = 0
            for t in range(NT):
                if t % 4 == 0:
                    x6_tiles[t // 4 + 2] = issue_x6_load(t // 4 + 2)
                if t % 2 == 0:
                    wts[t // 2 + 2] = issue_wt_load(t // 2 + 2)
                fb4s[t + 1] = feature_stage(t + 1, x6_tiles)
                fb4 = fb4s.pop(t)
                wt = wts[t // 2]
                if t % 2 == 1:
                    wts.pop(t // 2)
                iw = t * 4
                st0 = stagep.tile([128, 4 * D], bf16, tag="stage")
                st1 = stagep.tile([128, 4 * D], bf16, tag="stage")
                sts = (st0, st1)
                for j2 in range(2):
                    for c in range(2):
                        po = pop.tile([128, 2 * D], f32)
                        for h in range(2):
                            j = 2 * j2 + h
                            u, cb = j % 2, (j // 2) * 256
                            nc.tensor.matmul(
                                po[:, h * D:(h + 1) * D],
                                fb4[64 * u:64 * u + KW,
                                    cb + c * 128:cb + c * 128 + 128],
                                wt[64 * u:64 * u + KW,
                                   ((t % 2) * 2 + j // 2) * D:
                                   ((t % 2) * 2 + j // 2 + 1) * D],
                                start=True,
                                stop=True,
                            )
                        dst = sts[c][:, 2 * j2 * D:(2 * j2 + 2) * D]
                        # keep ACT(exp+copies) ~= DVE(copies)
                        use_act = ((ncopy + 1) * _NACT) // 256 > nact_done
                        ncopy += 1
                        if use_act:
                            nact_done += 1
                            nc.scalar.copy(dst, po[:, :])
                        else:
                            nc.vector.tensor_scalar_mul(dst, po[:, :], 1.0)
                        if j2 == 1:
                            eng_dma = nc.sync if c == 0 else nc.gpsimd
                            eng_dma.dma_start(
                                out=out[c * 128:(c + 1) * 128, iw:iw + 4, :],
                                in_=sts[c][:, :],
                            )

    _split_multi_waits(nc)
    return nc


def _split_multi_waits(nc):
    """Walrus TPB instruction structs accept a single sync wait. Hoist all
    but the last wait of any instruction onto same-engine NOPs inserted
    immediately before it (a wait executes before the instruction either
    way, so this is semantically identical)."""
    import dataclasses
    import concourse.bass as bass
    import concourse.mybir as mybir

    tpl = bass.Bass().sync.nop().ins
    k = 0
    for blk in nc.m.functions[0].blocks:
        out_insts = []
        for inst in blk.instructions:
            si = getattr(inst, "sync_info", None)
            if si is not None and len(si.on_wait) > 1:
                for w in si.on_wait[:-1]:
                    out_insts.append(
                        dataclasses.replace(
                            tpl,
                            name=f"nop-w{k}",
                            engine=inst.engine,
                            sync_info=mybir.SyncInfo(on_wait=[w], on_update=[]),
                        )
                    )
                    k += 1
                inst.sync_info = dataclasses.replace(si, on_wait=si.on_wait[-1:])
            out_insts.append(inst)
        blk.instructions[:] = out_insts


def _hi_lo(v, bf16):
    """Double-bf16 split: v ~= hi + lo with |err| <~ |v| * 2^-17."""
    hi = v.astype(bf16)
    lo = (v - hi.astype(np.float32)).astype(bf16)
    return hi, lo


def _fit_basis(x, grid):
    """Ridge-fit the projection T (M x G) of the grid Gaussians onto M
    coarse sigma-1 Gaussians, weighted by the empirical x distribution."""
    xf = x.ravel().astype(np.float64)
    xs_emp = xf[::17][:30000]
    lo, hi = xf.min() - 0.3, xf.max() + 0.3
    xs_uni = np.linspace(lo, hi, 4000)
    xs = np.concatenate([xs_emp, xs_uni])
    w_s = np.concatenate(
        [np.full(xs_emp.size, 1.0), np.full(xs_uni.size, 0.05)]
    )
    g64 = grid.astype(np.float64)
    ctr = np.linspace(g64.min() - 0.5, g64.max() + 0.5, M)
    PHI = np.exp(-0.5 * (xs[:, None] - g64[None, :]) ** 2)
    PSI = np.exp(-0.5 * (xs[:, None] - ctr[None, :]) ** 2)
    Aw = PSI * np.sqrt(w_s)[:, None]
    Bw = PHI * np.sqrt(w_s)[:, None]
    T = np.linalg.solve(Aw.T @ Aw + 1e-4 * np.eye(M), Aw.T @ Bw)
    return ctr, T


def _prep_inputs(x, base_weight, gp_weight, grid):
    import ml_dtypes

    bf16 = ml_dtypes.bfloat16
    x = np.ascontiguousarray(np.asarray(x, np.float32))
    base_weight = np.asarray(base_weight, np.float32)
    gp_weight = np.asarray(gp_weight, np.float32)
    grid = np.asarray(grid, np.float32)

    ctr, T = _fit_basis(x, grid)

    # x6 feature order: per tile of 4, strip u holds (f+u, f+u+2)
    permx = np.empty(NF, np.int64)
    k = 0
    for t in range(NT):
        for u in range(2):
            permx[k] = t * 4 + u
            permx[k + 1] = t * 4 + u + 2
            k += 2

    # wcatT feature order: per group of 8 (2 tiles), band u holds
    # [f+u, f+u+2, f+4+u, f+4+u+2] as one contiguous 4D-column span
    permw = np.empty(NF, np.int64)
    k = 0
    for g in range(NT // 2):
        f = g * 8
        for u in range(2):
            for v in (0, 2, 4, 6):
                permw[k] = f + u + v
                k += 1

    # c[m, i, d] = sum_g T[m, g] gp_w[i, g, d]; row M = base_weight
    cw = np.einsum("mg,igd->mid", T, gp_weight.astype(np.float64))
    wcat = np.concatenate([cw, base_weight[None].astype(np.float64)], axis=0)
    wcatT = np.ascontiguousarray(
        wcat[:, permw, :].reshape(KW, NF * D).astype(bf16)
    )

    h_hi, h_lo = _hi_lo(ctr.astype(np.float32), bf16)
    # stationary (5, 256): 2 blocks of (5, 128); block u's active cols sit
    # at offset 64u so strip u's args land at PSUM partitions 64u..64u+M-1
    s5q = np.zeros((KF, 256), bf16)
    for u in range(2):
        o = u * 128 + 64 * u
        s5q[0, o:o + M] = np.ones(M, bf16)
        s5q[1, o:o + M] = np.ones(M, bf16)
        s5q[2, o:o + M] = h_hi
        s5q[3, o:o + M] = h_lo
        s5q[4, o:o + M] = h_hi
    s5q = np.ascontiguousarray(s5q)

    biasv = np.full((128, 1), -30.0, np.float32)
    for u in range(2):
        biasv[64 * u:64 * u + M, 0] = (-0.5 * ctr * ctr).astype(np.float32)
    biasv = np.ascontiguousarray(biasv)

    in_maps = []
    for cidx in range(NCORES):
        xT = np.ascontiguousarray(x[cidx * BL:(cidx + 1) * BL, :].T)  # (NF, BL)
        xTp = xT[permx]
        mx2 = (-0.5 * xTp * xTp).ravel()
        xr = xTp.ravel()
        mx2_hi, mx2_lo = _hi_lo(mx2, bf16)
        x_hi, x_lo = _hi_lo(xr, bf16)
        silu = (xTp / (1.0 + np.exp(-xTp))).ravel().astype(bf16)
        x6 = np.ascontiguousarray(
            np.stack([mx2_hi, mx2_lo, x_hi, x_hi, x_lo, silu])
        )
        in_maps.append(
            {"x6": x6, "wcatT": wcatT, "s5q": s5q, "biasv": biasv}
        )
    return in_maps


def _run(in_maps, **kw):
    from concourse.bass_utils import run_bass_kernel_spmd

    if "nc" not in _cache:
        _cache["nc"] = _build()
    return run_bass_kernel_spmd(_cache["nc"], in_maps, list(range(NCORES)), **kw)


def kernel(x, base_weight, gp_weight, grid):
    in_maps = _prep_inputs(x, base_weight, gp_weight, grid)
    res = _run(in_maps)
    return np.concatenate(
        [np.asarray(r["out"]).astype(np.float32) for r in res.results], axis=0
    )


# revision 17
# speedup vs baseline: 1.0552x; 1.0552x over previous
"""KAN embeddings Bass kernel for Trainium2, 8-core data-parallel over batch.

out[b,i,d] = silu(x[b,i]) * base_w[i,d] + sum_g exp(-0.5(x[b,i]-grid[g])^2) * gp_w[i,g,d]

v2.1 strategy: the 64 grid Gaussians (sigma=1, spacing 0.063) are massively
oversampled -- the gp branch is re-expressed in a coarse basis of M=12
sigma-1 Gaussians with centers spanning [grid_min-0.5, grid_max+0.5].
The projection T (ridge least-squares fit on the empirical x distribution,
done per-call on host) folds into the weights: c[m,i,d] = sum_g T[m,g]
gp_w[i,g,d]. Validated offline: rel err 0.0025 vs 0.0027 for the exact-
basis bf16 path, because |c| stays O(1). Weight DMA drops 17 MB -> 3.4 MB
per core; the kernel becomes output-store-bound (~67 MB bf16 out).

Layout per core (batch shard BL=256), per tile of 4 features, 64 tiles:
  - fb16 quad tile (128, 2048) covers 4 tiles; per tile a (128, 512) slice:
    strip u (u=0,1; PE operand bases must be 0/32/64) rows 64u..64u+11 =
    coarse RBF feats, row 64u+12 = silu; cols = feature (f+u)'s 256 batch
    ++ feature (f+u+2)'s 256 batch. Pad rows hold exp(-30)~1e-13 junk.
  - RBF args via K=5 broadcast matmuls (double-bf16 split of -x^2/2 and x)
    with 2 zero-padded stationaries (5,128) accumulating into ONE (128,512)
    PSUM bank; ONE exp ACT pass per tile (bias -h^2/2 per partition, -30 on
    pad rows). ACT cost is free-size-bound, so strip-stacking halves the
    exp instruction count vs (64,512)-tile v1.
  - silu rows: ONE batched DMA per quad ((2,2048), partitions {12,76}) on
    the ACT HWDGE ring right after the quad's last exp -- same-engine
    program order gives the WAW-after-exp ordering for free, and batching
    keeps the ~0.6us/dma engine-issue cost off the ACT budget.
  - main matmuls MUST be full-height (128,128) stationaries: fast weight
    load only engages for 128-row stationaries (measured 375 vs 730 ns per
    N=512 matmul). Stationary = fb16[0:128, feature cols]; the strip
    structure is compensated in the moving operand: strip-0 features'
    weights live in wtA tiles (rows 0..12 = data, rows 13..127 zeroed
    once), strip-1 features' in wtB tiles (rows 64..76 = data, rest zeroed
    once). Zero regions never overlap the per-group load bands, so weight
    loads never wait on the one-time gpsimd memsets, and fb16 pad junk
    (~1e-13) always multiplies exact zeros.
  - PSUM -> SBUF copies (only ACT and DVE read PSUM) drain 2-bank
    128x1024 tiles, downcast to bf16, balanced so ACT(exp+silu+copies) ~=
    DVE(copies); output DMAs are bf16, issued per (tile, batch-half);
    c=0 stores on the sync HWDGE ring, c=1 stores on gpsimd SWDGE
    (latency-tolerant). Host upcasts bf16 -> f32.
"""

import numpy as np

B, NF, G, D = 2048, 256, 64, 512
NCORES = 8
BL = B // NCORES          # 256 batch rows per core
M = 12                    # coarse Gaussian basis size
KW = M + 1                # weight rows: M coarse + base row
NT = NF // 4              # 64 tiles of 4 features
KF = 5                    # feature-matmul contraction rows

# ACT copies out of 256 total so ACT(exp+silu_dma+copies) ~ DVE(copies)
_NACT = 103

_cache = {}


def _build():
    import concourse.bass as bass
    from concourse import mybir
    from concourse import tile

    f32 = mybir.dt.float32
    bf16 = mybir.dt.bfloat16
    AF = mybir.ActivationFunctionType

    nc = bass.Bass()
    x6 = nc.declare_dram_parameter("x6", [KF + 1, NF * BL], bf16, isOutput=False)
    wcatT = nc.declare_dram_parameter("wcatT", [KW, NF * D], bf16, isOutput=False)
    s5q = nc.declare_dram_parameter("s5q", [KF, 256], bf16, isOutput=False)
    biasv = nc.declare_dram_parameter("biasv", [128, 1], f32, isOutput=False)
    zpad = nc.declare_dram_parameter("zpad", [32 - KW, 4 * D], bf16, isOutput=False)
    out = nc.declare_dram_parameter("out", [BL, NF, D], bf16, isOutput=True)

    with tile.TileContext(nc) as tc:
        with (
            tc.tile_pool(name="const", bufs=1) as constp,
            tc.tile_pool(name="x6p", bufs=3) as x6p,
            tc.tile_pool(name="fbqp", bufs=3) as fbqp,
            tc.tile_pool(name="stage", bufs=6) as stagep,
            tc.tile_pool(name="pt", bufs=2, space="PSUM") as ptp,
            tc.tile_pool(name="po", bufs=3, space="PSUM") as pop,
        ):
            s5q_t = constp.tile([KF, 256], bf16)
            nc.sync.dma_start(out=s5q_t[:, :], in_=s5q[:, :])
            biasv_t = constp.tile([128, 1], f32)
            nc.sync.dma_start(out=biasv_t[:, :], in_=biasv[:, :])

            # Persistent weight slots: full-height moving operands with
            # one-time-zeroed pad rows. Strip-0 features in A (data rows
            # 0..KW-1), strip-1 in B (data rows 64..64+KW-1). Memset
            # regions never overlap the load bands, so loads don't wait.
            # Order A0,B0,A1,B1,... so zeroing finishes just ahead of each
            # group's first main matmul; gpsimd's c=1 stores queue behind
            # but are latency-tolerant.
            wtA_slots = [
                constp.tile([128, 4 * D], bf16, name=f"wtA{i}") for i in range(4)
            ]
            wtB_slots = [
                constp.tile([128, 4 * D], bf16, name=f"wtB{i}") for i in range(4)
            ]
            # engine memsets need 32-aligned partition bases; the unaligned
            # remainders (rows KW..31 / 64+KW..95) are zero-filled by small
            # DMAs from a DRAM zeros block (DMA APs are partition-arbitrary)
            for i in range(4):
                nc.gpsimd.memset(wtA_slots[i][32:64, :], 0.0)
                nc.gpsimd.memset(wtA_slots[i][64:128, :], 0.0)
                nc.sync.dma_start(out=wtA_slots[i][KW:32, :], in_=zpad[:, :])
                nc.gpsimd.memset(wtB_slots[i][0:64, :], 0.0)
                nc.gpsimd.memset(wtB_slots[i][96:128, :], 0.0)
                nc.sync.dma_start(out=wtB_slots[i][64 + KW:96, :], in_=zpad[:, :])

            # x6 loads cover 4 tiles each (5, 4096); the silu row (row 5)
            # goes straight DRAM -> fb16 and is never staged
            def issue_x6_load(q):
                if q >= NT // 4:
                    return None
                x6_t = x6p.tile([KF, 4096], bf16)
                nc.sync.dma_start(
                    out=x6_t[:, :], in_=x6[0:KF, q * 4096:(q + 1) * 4096]
                )
                return x6_t

            # weight loads for group g (2 tiles, 8 features): wcatT is
            # host-permuted so band A = [f,f+2,f+4,f+6], band B =
            # [f+1,f+3,f+5,f+7] are contiguous 4D-column spans
            def issue_wt_load(g):
                if g >= NT // 2:
                    return None
                wtA, wtB = wtA_slots[g % 4], wtB_slots[g % 4]
                nc.sync.dma_start(
                    out=wtA[0:KW, :],
                    in_=wcatT[0:KW, (g * 8) * D:(g * 8 + 4) * D],
                )
                nc.sync.dma_start(
                    out=wtB[64:64 + KW, :],
                    in_=wcatT[0:KW, (g * 8 + 4) * D:(g * 8 + 8) * D],
                )
                return wtA, wtB

            # feature stage for tile t: 2 accumulating matmuls -> 1 exp into
            # the quad tile; after the quad's last exp, ONE batched silu DMA
            def feature_stage(t, x6_tiles, fbqs):
                if t >= NT:
                    return
                q, tl = t // 4, t % 4
                if tl == 0:
                    fbqs[q] = fbqp.tile([128, 2048], bf16, name=f"fbq{q}", tag="fbq")
                fbq = fbqs[q]
                x6_t = x6_tiles[q]
                off = tl * 1024
                pt = ptp.tile([128, 512], f32)
                for u in range(2):
                    nc.tensor.matmul(
                        pt[:, :],
                        s5q_t[0:KF, u * 128:(u + 1) * 128],
                        x6_t[0:KF, off + u * 512:off + (u + 1) * 512],
                        start=(u == 0),
                        stop=(u == 1),
                    )
                nc.scalar.activation(
                    fbq[:, tl * 512:(tl + 1) * 512], pt[:, :], AF.Exp,
                    bias=biasv_t[:, :], scale=1.0,
                )
                if tl == 3:
                    silu_dst = fbq[:, :].rearrange(
                        "(u p) (t c) -> u p t c", u=2, t=4
                    )[:, 12, :, :]
                    silu_src = x6[KF:KF + 1, q * 4096:(q + 1) * 4096].rearrange(
                        "one (t u c) -> (one u) t c", t=4, u=2
                    )
                    nc.scalar.dma_start(out=silu_dst, in_=silu_src)

            x6_tiles = {0: issue_x6_load(0), 1: issue_x6_load(1)}
            wts = {0: issue_wt_load(0), 1: issue_wt_load(1)}
            fbqs = {}
            for t0 in range(4):
                feature_stage(t0, x6_tiles, fbqs)

            ncopy = 0
            nact_done = 0
            for t in range(NT):
                if t % 4 == 0:
                    x6_tiles[t // 4 + 2] = issue_x6_load(t // 4 + 2)
                if t % 2 == 0:
                    wts[t // 2 + 2] = issue_wt_load(t // 2 + 2)
                feature_stage(t + 4, x6_tiles, fbqs)
                q, tl = t // 4, t % 4
                fbq = fbqs[q]
                if tl == 3:
                    fbqs.pop(q)
                wtA, wtB = wts[t // 2]
                if t % 2 == 1:
                    wts.pop(t // 2)
                iw = t * 4
                st0 = stagep.tile([128, 4 * D], bf16, tag="stage")
                st1 = stagep.tile([128, 4 * D], bf16, tag="stage")
                sts = (st0, st1)
                for j2 in range(2):
                    for c in range(2):
                        po = pop.tile([128, 2 * D], f32)
                        for h in range(2):
                            j = 2 * j2 + h
                            u, cb = j % 2, (j // 2) * 256
                            wt = wtA if u == 0 else wtB
                            nc.tensor.matmul(
                                po[:, h * D:(h + 1) * D],
                                fbq[0:128,
                                    tl * 512 + cb + c * 128:
                                    tl * 512 + cb + c * 128 + 128],
                                wt[0:128,
                                   ((t % 2) * 2 + j // 2) * D:
                                   ((t % 2) * 2 + j // 2 + 1) * D],
                                start=True,
                                stop=True,
                            )
                        dst = sts[c][:, 2 * j2 * D:(2 * j2 + 2) * D]
                        # keep ACT(exp+silu+copies) ~= DVE(copies)
                        use_act = ((ncopy + 1) * _NACT) // 256 > nact_done
                        ncopy += 1
                        if use_act:
                            nact_done += 1
                            nc.scalar.copy(dst, po[:, :])
                        else:
                            nc.vector.tensor_scalar_mul(dst, po[:, :], 1.0)
                        if j2 == 1:
                            eng_dma = nc.sync if c == 0 else nc.gpsimd
                            eng_dma.dma_start(
                                out=out[c * 128:(c + 1) * 128, iw:iw + 4, :],
                                in_=sts[c][:, :],
                            )

    _split_multi_waits(nc)
    return nc


def _split_multi_waits(nc):
    """Walrus TPB instruction structs accept a single sync wait. Hoist all
    but the last wait of any instruction onto same-engine NOPs inserted
    immediately before it (a wait executes before the instruction either
    way, so this is semantically identical)."""
    import dataclasses
    import concourse.bass as bass
    import concourse.mybir as mybir

    tpl = bass.Bass().sync.nop().ins
    k = 0
    for blk in nc.m.functions[0].blocks:
        out_insts = []
        for inst in blk.instructions:
            si = getattr(inst, "sync_info", None)
            if si is not None and len(si.on_wait) > 1:
                for w in si.on_wait[:-1]:
                    out_insts.append(
                        dataclasses.replace(
                            tpl,
                            name=f"nop-w{k}",
                            engine=inst.engine,
                            sync_info=mybir.SyncInfo(on_wait=[w], on_update=[]),
                        )
                    )
                    k += 1
                inst.sync_info = dataclasses.replace(si, on_wait=si.on_wait[-1:])
            out_insts.append(inst)
        blk.instructions[:] = out_insts


def _hi_lo(v, bf16):
    """Double-bf16 split: v ~= hi + lo with |err| <~ |v| * 2^-17."""
    hi = v.astype(bf16)
    lo = (v - hi.astype(np.float32)).astype(bf16)
    return hi, lo


def _fit_basis(x, grid):
    """Ridge-fit the projection T (M x G) of the grid Gaussians onto M
    coarse sigma-1 Gaussians, weighted by the empirical x distribution."""
    xf = x.ravel().astype(np.float64)
    xs_emp = xf[::17][:30000]
    lo, hi = xf.min() - 0.3, xf.max() + 0.3
    xs_uni = np.linspace(lo, hi, 4000)
    xs = np.concatenate([xs_emp, xs_uni])
    w_s = np.concatenate(
        [np.full(xs_emp.size, 1.0), np.full(xs_uni.size, 0.05)]
    )
    g64 = grid.astype(np.float64)
    ctr = np.linspace(g64.min() - 0.5, g64.max() + 0.5, M)
    PHI = np.exp(-0.5 * (xs[:, None] - g64[None, :]) ** 2)
    PSI = np.exp(-0.5 * (xs[:, None] - ctr[None, :]) ** 2)
    Aw = PSI * np.sqrt(w_s)[:, None]
    Bw = PHI * np.sqrt(w_s)[:, None]
    T = np.linalg.solve(Aw.T @ Aw + 1e-4 * np.eye(M), Aw.T @ Bw)
    return ctr, T


def _prep_inputs(x, base_weight, gp_weight, grid):
    import ml_dtypes

    bf16 = ml_dtypes.bfloat16
    x = np.ascontiguousarray(np.asarray(x, np.float32))
    base_weight = np.asarray(base_weight, np.float32)
    gp_weight = np.asarray(gp_weight, np.float32)
    grid = np.asarray(grid, np.float32)

    ctr, T = _fit_basis(x, grid)

    # x6 feature order: per tile of 4, strip u holds (f+u, f+u+2)
    permx = np.empty(NF, np.int64)
    k = 0
    for t in range(NT):
        for u in range(2):
            permx[k] = t * 4 + u
            permx[k + 1] = t * 4 + u + 2
            k += 2

    # wcatT feature order: per group of 8 (2 tiles), band A holds
    # [f, f+2, f+4, f+6], band B [f+1, f+3, f+5, f+7], each contiguous
    permw = np.empty(NF, np.int64)
    k = 0
    for g in range(NT // 2):
        f = g * 8
        for u in range(2):
            for v in (0, 2, 4, 6):
                permw[k] = f + u + v
                k += 1

    # c[m, i, d] = sum_g T[m, g] gp_w[i, g, d]; row M = base_weight
    cw = np.einsum("mg,igd->mid", T, gp_weight.astype(np.float64))
    wcat = np.concatenate([cw, base_weight[None].astype(np.float64)], axis=0)
    wcatT = np.ascontiguousarray(
        wcat[:, permw, :].reshape(KW, NF * D).astype(bf16)
    )

    h_hi, h_lo = _hi_lo(ctr.astype(np.float32), bf16)
    # stationary (5, 256): 2 blocks of (5, 128); block u's active cols sit
    # at offset 64u so strip u's args land at PSUM partitions 64u..64u+M-1
    s5q = np.zeros((KF, 256), bf16)
    for u in range(2):
        o = u * 128 + 64 * u
        s5q[0, o:o + M] = np.ones(M, bf16)
        s5q[1, o:o + M] = np.ones(M, bf16)
        s5q[2, o:o + M] = h_hi
        s5q[3, o:o + M] = h_lo
        s5q[4, o:o + M] = h_hi
    s5q = np.ascontiguousarray(s5q)

    biasv = np.full((128, 1), -30.0, np.float32)
    for u in range(2):
        biasv[64 * u:64 * u + M, 0] = (-0.5 * ctr * ctr).astype(np.float32)
    biasv = np.ascontiguousarray(biasv)

    in_maps = []
    for cidx in range(NCORES):
        xT = np.ascontiguousarray(x[cidx * BL:(cidx + 1) * BL, :].T)  # (NF, BL)
        xTp = xT[permx]
        mx2 = (-0.5 * xTp * xTp).ravel()
        xr = xTp.ravel()
        mx2_hi, mx2_lo = _hi_lo(mx2, bf16)
        x_hi, x_lo = _hi_lo(xr, bf16)
        silu = (xTp / (1.0 + np.exp(-xTp))).ravel().astype(bf16)
        x6 = np.ascontiguousarray(
            np.stack([mx2_hi, mx2_lo, x_hi, x_hi, x_lo, silu])
        )
        in_maps.append(
            {"x6": x6, "wcatT": wcatT, "s5q": s5q, "biasv": biasv,
             "zpad": np.zeros((32 - KW, 4 * D), bf16)}
        )
    return in_maps


def _run(in_maps, **kw):
    from concourse.bass_utils import run_bass_kernel_spmd

    if "nc" not in _cache:
        _cache["nc"] = _build()
    return run_bass_kernel_spmd(_cache["nc"], in_maps, list(range(NCORES)), **kw)


def kernel(x, base_weight, gp_weight, grid):
    in_maps = _prep_inputs(x, base_weight, gp_weight, grid)
    res = _run(in_maps)
    return np.concatenate(
        [np.asarray(r["out"]).astype(np.float32) for r in res.results], axis=0
    )


# revision 23
# speedup vs baseline: 1.2582x; 1.1924x over previous
"""KAN embeddings Bass kernel for Trainium2, 8-core data-parallel over batch.

out[b,i,d] = silu(x[b,i]) * base_w[i,d] + sum_g exp(-0.5(x[b,i]-grid[g])^2) * gp_w[i,g,d]

v2.1 strategy: the 64 grid Gaussians (sigma=1, spacing 0.063) are massively
oversampled -- the gp branch is re-expressed in a coarse basis of M=12
sigma-1 Gaussians with centers spanning [grid_min-0.5, grid_max+0.5].
The projection T (ridge least-squares fit on the empirical x distribution,
done per-call on host) folds into the weights: c[m,i,d] = sum_g T[m,g]
gp_w[i,g,d]. Validated offline: rel err 0.0025 vs 0.0027 for the exact-
basis bf16 path, because |c| stays O(1). Weight DMA drops 17 MB -> 3.4 MB
per core; the kernel becomes output-store-bound (~67 MB bf16 out).

Layout per core (batch shard BL=256), per tile of 4 features, 64 tiles:
  - fb16 quad tile (128, 2048) covers 4 tiles; per tile a (128, 512) slice:
    strip u (u=0,1; PE operand bases must be 0/32/64) rows 64u..64u+11 =
    coarse RBF feats, row 64u+12 = silu; cols = feature (f+u)'s 256 batch
    ++ feature (f+u+2)'s 256 batch. Pad rows hold exp(-30)~1e-13 junk.
  - RBF args via K=5 broadcast matmuls (double-bf16 split of -x^2/2 and x)
    with 2 zero-padded stationaries (5,128) accumulating into ONE (128,512)
    PSUM bank; ONE exp ACT pass per tile (bias -h^2/2 per partition, -30 on
    pad rows). ACT cost is free-size-bound, so strip-stacking halves the
    exp instruction count vs (64,512)-tile v1.
  - silu rows: ONE batched DMA per quad ((2,2048), partitions {12,76}) on
    the ACT HWDGE ring right after the quad's last exp -- same-engine
    program order gives the WAW-after-exp ordering for free, and batching
    keeps the ~0.6us/dma engine-issue cost off the ACT budget.
  - main matmuls MUST be full-height (128,128) stationaries: fast weight
    load only engages for 128-row stationaries (measured 375 vs 730 ns per
    N=512 matmul). Stationary = fb16[0:128, feature cols]; the strip
    structure is compensated in the moving operand: strip-0 features'
    weights live in wtA tiles (rows 0..12 = data, rows 13..127 zeroed
    once), strip-1 features' in wtB tiles (rows 64..76 = data, rest zeroed
    once). Zero regions never overlap the per-group load bands, so weight
    loads never wait on the one-time gpsimd memsets, and fb16 pad junk
    (~1e-13) always multiplies exact zeros.
  - PSUM -> SBUF copies (only ACT and DVE read PSUM) drain 2-bank
    128x1024 tiles, downcast to bf16, balanced so ACT(exp+silu+copies) ~=
    DVE(copies); output DMAs are bf16, issued per (tile, batch-half);
    c=0 stores on the sync HWDGE ring, c=1 stores on gpsimd SWDGE
    (latency-tolerant). Host upcasts bf16 -> f32.
"""

import numpy as np

B, NF, G, D = 2048, 256, 64, 512
NCORES = 8
BL = B // NCORES          # 256 batch rows per core
M = 12                    # coarse Gaussian basis size
KW = M + 1                # weight rows: M coarse + base row
NT = NF // 4              # 64 tiles of 4 features
KF = 5                    # feature-matmul contraction rows

# ACT copies out of 256 total so ACT(exp+silu_dma+copies) ~ DVE(copies)
_NACT = 111

_cache = {}


def _build():
    import concourse.bass as bass
    from concourse import mybir
    from concourse import tile

    f32 = mybir.dt.float32
    bf16 = mybir.dt.bfloat16
    AF = mybir.ActivationFunctionType

    nc = bass.Bass()
    x10 = nc.declare_dram_parameter("x10", [2 * KF, NF * BL // 2], bf16, isOutput=False)
    xsilu = nc.declare_dram_parameter("xsilu", [1, NF * BL], bf16, isOutput=False)
    wcatT = nc.declare_dram_parameter("wcatT", [KW, NF * D], bf16, isOutput=False)
    s10 = nc.declare_dram_parameter("s10", [2 * KF, 128], bf16, isOutput=False)
    biasv = nc.declare_dram_parameter("biasv", [128, 1], f32, isOutput=False)
    zpad = nc.declare_dram_parameter("zpad", [32 - KW, 4 * D], bf16, isOutput=False)
    out = nc.declare_dram_parameter("out", [BL, NF, D], bf16, isOutput=True)

    with tile.TileContext(nc) as tc:
        with (
            tc.tile_pool(name="const", bufs=1) as constp,
            tc.tile_pool(name="x10p", bufs=4) as x10p,
            tc.tile_pool(name="fbqp", bufs=4) as fbqp,
            tc.tile_pool(name="stage", bufs=6) as stagep,
            tc.tile_pool(name="pt", bufs=2, space="PSUM") as ptp,
            tc.tile_pool(name="po", bufs=3, space="PSUM") as pop,
        ):
            s10_t = constp.tile([2 * KF, 128], bf16)
            nc.sync.dma_start(out=s10_t[:, :], in_=s10[:, :])
            biasv_t = constp.tile([128, 1], f32)
            nc.sync.dma_start(out=biasv_t[:, :], in_=biasv[:, :])

            # Persistent weight slots: full-height moving operands with
            # one-time-zeroed pad rows. Strip-0 features in A (data rows
            # 0..KW-1), strip-1 in B (data rows 64..64+KW-1). Zero regions
            # never overlap the load bands, so loads don't wait. Bulk zeros
            # split vector (slots 0,1) / gpsimd (slots 2,3) so each slot is
            # ready just ahead of its group's first main matmul; unaligned
            # 19-row slivers come from a DRAM zeros block on the gpsimd
            # ring (latency-tolerant, keeps the sync ring free for the
            # critical x10/weight loads at startup).
            wtA_slots = [
                constp.tile([128, 4 * D], bf16, name=f"wtA{i}") for i in range(4)
            ]
            wtB_slots = [
                constp.tile([128, 4 * D], bf16, name=f"wtB{i}") for i in range(4)
            ]

            def init_wt_slot(i, eng):
                eng.memset(wtA_slots[i][32:64, :], 0.0)
                eng.memset(wtA_slots[i][64:128, :], 0.0)
                eng.memset(wtB_slots[i][0:64, :], 0.0)
                eng.memset(wtB_slots[i][96:128, :], 0.0)

            for i in range(4):
                nc.gpsimd.dma_start(out=wtA_slots[i][KW:32, :], in_=zpad[:, :])
                nc.gpsimd.dma_start(
                    out=wtB_slots[i][64 + KW:96, :], in_=zpad[:, :]
                )
            init_wt_slot(0, nc.vector)
            init_wt_slot(1, nc.vector)
            init_wt_slot(2, nc.gpsimd)
            init_wt_slot(3, nc.gpsimd)

            # x10 loads cover 4 tiles each (10, 2048); the silu rows go
            # straight DRAM -> fb16 and are never staged
            def issue_x10_load(q):
                if q >= NT // 4:
                    return None
                x10_t = x10p.tile([2 * KF, 2048], bf16)
                nc.sync.dma_start(
                    out=x10_t[:, :], in_=x10[0:2 * KF, q * 2048:(q + 1) * 2048]
                )
                return x10_t

            # weight loads for group g (2 tiles, 8 features): wcatT is
            # host-permuted so band A = [f,f+2,f+4,f+6], band B =
            # [f+1,f+3,f+5,f+7] are contiguous 4D-column spans
            def issue_wt_load(g):
                if g >= NT // 2:
                    return None
                wtA, wtB = wtA_slots[g % 4], wtB_slots[g % 4]
                nc.sync.dma_start(
                    out=wtA[0:KW, :],
                    in_=wcatT[0:KW, (g * 8) * D:(g * 8 + 4) * D],
                )
                nc.sync.dma_start(
                    out=wtB[64:64 + KW, :],
                    in_=wcatT[0:KW, (g * 8 + 4) * D:(g * 8 + 8) * D],
                )
                return wtA, wtB

            # feature stage for tile t: ONE K=10 matmul (both strips' args
            # via a double-height stationary) -> 1 exp into the quad tile;
            # after the quad's last exp, ONE batched silu DMA
            def feature_stage(t, x10_tiles, fbqs):
                if t >= NT:
                    return
                q, tl = t // 4, t % 4
                if tl == 0:
                    fbqs[q] = fbqp.tile([128, 2048], bf16, name=f"fbq{q}", tag="fbq")
                fbq = fbqs[q]
                x10_t = x10_tiles[q]
                pt = ptp.tile([128, 512], f32)
                nc.tensor.matmul(
                    pt[:, :],
                    s10_t[0:2 * KF, 0:128],
                    x10_t[0:2 * KF, tl * 512:(tl + 1) * 512],
                    start=True,
                    stop=True,
                )
                nc.scalar.activation(
                    fbq[:, tl * 512:(tl + 1) * 512], pt[:, :], AF.Exp,
                    bias=biasv_t[:, :], scale=1.0,
                )
                if tl == 3:
                    silu_dst = fbq[:, :].rearrange(
                        "(u p) (t c) -> u p t c", u=2, t=4
                    )[:, 12, :, :]
                    silu_src = xsilu[0:1, q * 4096:(q + 1) * 4096].rearrange(
                        "one (t u c) -> (one u) t c", t=4, u=2
                    )
                    nc.scalar.dma_start(out=silu_dst, in_=silu_src)

            x10_tiles = {q: issue_x10_load(q) for q in range(3)}
            wts = {0: issue_wt_load(0), 1: issue_wt_load(1)}
            fbqs = {}
            for t0 in range(8):
                feature_stage(t0, x10_tiles, fbqs)

            ncopy = 0
            nact_done = 0
            for t in range(NT):
                if t % 4 == 0:
                    x10_tiles[t // 4 + 3] = issue_x10_load(t // 4 + 3)
                if t % 2 == 0:
                    wts[t // 2 + 2] = issue_wt_load(t // 2 + 2)
                feature_stage(t + 8, x10_tiles, fbqs)
                q, tl = t // 4, t % 4
                fbq = fbqs[q]
                if tl == 3:
                    fbqs.pop(q)
                wtA, wtB = wts[t // 2]
                if t % 2 == 1:
                    wts.pop(t // 2)
                iw = t * 4
                st0 = stagep.tile([128, 4 * D], bf16, tag="stage")
                st1 = stagep.tile([128, 4 * D], bf16, tag="stage")
                sts = (st0, st1)
                for j2 in range(2):
                    for c in range(2):
                        po = pop.tile([128, 2 * D], f32)
                        for h in range(2):
                            j = 2 * j2 + h
                            u, cb = j % 2, (j // 2) * 256
                            wt = wtA if u == 0 else wtB
                            nc.tensor.matmul(
                                po[:, h * D:(h + 1) * D],
                                fbq[0:128,
                                    tl * 512 + cb + c * 128:
                                    tl * 512 + cb + c * 128 + 128],
                                wt[0:128,
                                   ((t % 2) * 2 + j // 2) * D:
                                   ((t % 2) * 2 + j // 2 + 1) * D],
                                start=True,
                                stop=True,
                            )
                        dst = sts[c][:, 2 * j2 * D:(2 * j2 + 2) * D]
                        # keep ACT(exp+silu+copies) ~= DVE(copies)
                        use_act = ((ncopy + 1) * _NACT) // 256 > nact_done
                        ncopy += 1
                        if use_act:
                            nact_done += 1
                            nc.scalar.copy(dst, po[:, :])
                        else:
                            nc.vector.tensor_scalar_mul(dst, po[:, :], 1.0)
                        if j2 == 1:
                            eng_dma = nc.sync if c == 0 else nc.gpsimd
                            eng_dma.dma_start(
                                out=out[c * 128:(c + 1) * 128, iw:iw + 4, :],
                                in_=sts[c][:, :],
                            )

    _split_multi_waits(nc)
    return nc


def _split_multi_waits(nc):
    """Walrus TPB instruction structs accept a single sync wait. Hoist all
    but the last wait of any instruction onto same-engine NOPs inserted
    immediately before it (a wait executes before the instruction either
    way, so this is semantically identical)."""
    import dataclasses
    import concourse.bass as bass
    import concourse.mybir as mybir

    tpl = bass.Bass().sync.nop().ins
    k = 0
    for blk in nc.m.functions[0].blocks:
        out_insts = []
        for inst in blk.instructions:
            si = getattr(inst, "sync_info", None)
            if si is not None and len(si.on_wait) > 1:
                for w in si.on_wait[:-1]:
                    out_insts.append(
                        dataclasses.replace(
                            tpl,
                            name=f"nop-w{k}",
                            engine=inst.engine,
                            sync_info=mybir.SyncInfo(on_wait=[w], on_update=[]),
                        )
                    )
                    k += 1
                inst.sync_info = dataclasses.replace(si, on_wait=si.on_wait[-1:])
            out_insts.append(inst)
        blk.instructions[:] = out_insts


def _hi_lo(v, bf16):
    """Double-bf16 split: v ~= hi + lo with |err| <~ |v| * 2^-17."""
    hi = v.astype(bf16)
    lo = (v - hi.astype(np.float32)).astype(bf16)
    return hi, lo


def _fit_basis(x, grid):
    """Ridge-fit the projection T (M x G) of the grid Gaussians onto M
    coarse sigma-1 Gaussians, weighted by the empirical x distribution."""
    xf = x.ravel().astype(np.float64)
    xs_emp = xf[::17][:30000]
    lo, hi = xf.min() - 0.3, xf.max() + 0.3
    xs_uni = np.linspace(lo, hi, 4000)
    xs = np.concatenate([xs_emp, xs_uni])
    w_s = np.concatenate(
        [np.full(xs_emp.size, 1.0), np.full(xs_uni.size, 0.05)]
    )
    g64 = grid.astype(np.float64)
    ctr = np.linspace(g64.min() - 0.5, g64.max() + 0.5, M)
    PHI = np.exp(-0.5 * (xs[:, None] - g64[None, :]) ** 2)
    PSI = np.exp(-0.5 * (xs[:, None] - ctr[None, :]) ** 2)
    Aw = PSI * np.sqrt(w_s)[:, None]
    Bw = PHI * np.sqrt(w_s)[:, None]
    T = np.linalg.solve(Aw.T @ Aw + 1e-4 * np.eye(M), Aw.T @ Bw)
    return ctr, T


def _prep_inputs(x, base_weight, gp_weight, grid):
    import ml_dtypes

    bf16 = ml_dtypes.bfloat16
    x = np.ascontiguousarray(np.asarray(x, np.float32))
    base_weight = np.asarray(base_weight, np.float32)
    gp_weight = np.asarray(gp_weight, np.float32)
    grid = np.asarray(grid, np.float32)

    ctr, T = _fit_basis(x, grid)

    # x6 feature order: per tile of 4, strip u holds (f+u, f+u+2)
    permx = np.empty(NF, np.int64)
    k = 0
    for t in range(NT):
        for u in range(2):
            permx[k] = t * 4 + u
            permx[k + 1] = t * 4 + u + 2
            k += 2

    # wcatT feature order: per group of 8 (2 tiles), band A holds
    # [f, f+2, f+4, f+6], band B [f+1, f+3, f+5, f+7], each contiguous
    permw = np.empty(NF, np.int64)
    k = 0
    for g in range(NT // 2):
        f = g * 8
        for u in range(2):
            for v in (0, 2, 4, 6):
                permw[k] = f + u + v
                k += 1

    # c[m, i, d] = sum_g T[m, g] gp_w[i, g, d]; row M = base_weight
    cw = np.einsum("mg,igd->mid", T, gp_weight.astype(np.float64))
    wcat = np.concatenate([cw, base_weight[None].astype(np.float64)], axis=0)
    wcatT = np.ascontiguousarray(
        wcat[:, permw, :].reshape(KW, NF * D).astype(bf16)
    )

    h_hi, h_lo = _hi_lo(ctr.astype(np.float32), bf16)
    # double-height stationary (10, 128): rows 0..4 pair with strip-0
    # moving rows and activate cols 0..M-1 (PSUM partitions 0..M-1);
    # rows 5..9 pair with strip-1 moving rows, cols 64..64+M-1
    s10 = np.zeros((2 * KF, 128), bf16)
    for u in range(2):
        o = 64 * u
        r = KF * u
        s10[r + 0, o:o + M] = np.ones(M, bf16)
        s10[r + 1, o:o + M] = np.ones(M, bf16)
        s10[r + 2, o:o + M] = h_hi
        s10[r + 3, o:o + M] = h_lo
        s10[r + 4, o:o + M] = h_hi
    s10 = np.ascontiguousarray(s10)

    biasv = np.full((128, 1), -30.0, np.float32)
    for u in range(2):
        biasv[64 * u:64 * u + M, 0] = (-0.5 * ctr * ctr).astype(np.float32)
    biasv = np.ascontiguousarray(biasv)

    # even/odd feature splits per tile: strip 0 moving = [f, f+2],
    # strip 1 moving = [f+1, f+3]
    perm_ev = np.empty(NF // 2, np.int64)
    perm_od = np.empty(NF // 2, np.int64)
    k = 0
    for t in range(NT):
        perm_ev[k] = t * 4
        perm_ev[k + 1] = t * 4 + 2
        perm_od[k] = t * 4 + 1
        perm_od[k + 1] = t * 4 + 3
        k += 2

    def _x5_rows(xTs):
        mx2 = (-0.5 * xTs * xTs).ravel()
        xr = xTs.ravel()
        mx2_hi, mx2_lo = _hi_lo(mx2, bf16)
        x_hi, x_lo = _hi_lo(xr, bf16)
        return [mx2_hi, mx2_lo, x_hi, x_hi, x_lo]

    in_maps = []
    for cidx in range(NCORES):
        xT = np.ascontiguousarray(x[cidx * BL:(cidx + 1) * BL, :].T)  # (NF, BL)
        xTp = xT[permx]
        x10 = np.ascontiguousarray(
            np.stack(_x5_rows(xT[perm_ev]) + _x5_rows(xT[perm_od]))
        )
        xsilu = np.ascontiguousarray(
            (xTp / (1.0 + np.exp(-xTp))).ravel().astype(bf16).reshape(1, -1)
        )
        in_maps.append(
            {"x10": x10, "xsilu": xsilu, "wcatT": wcatT, "s10": s10,
             "biasv": biasv, "zpad": np.zeros((32 - KW, 4 * D), bf16)}
        )
    return in_maps


def _run(in_maps, **kw):
    from concourse.bass_utils import run_bass_kernel_spmd

    if "nc" not in _cache:
        _cache["nc"] = _build()
    return run_bass_kernel_spmd(_cache["nc"], in_maps, list(range(NCORES)), **kw)


def kernel(x, base_weight, gp_weight, grid):
    in_maps = _prep_inputs(x, base_weight, gp_weight, grid)
    res = _run(in_maps)
    return np.concatenate(
        [np.asarray(r["out"]).astype(np.float32) for r in res.results], axis=0
    )


# revision 28
# speedup vs baseline: 1.2586x; 1.0003x over previous
"""KAN embeddings Bass kernel for Trainium2, 8-core data-parallel over batch.

out[b,i,d] = silu(x[b,i]) * base_w[i,d] + sum_g exp(-0.5(x[b,i]-grid[g])^2) * gp_w[i,g,d]

v2.1 strategy: the 64 grid Gaussians (sigma=1, spacing 0.063) are massively
oversampled -- the gp branch is re-expressed in a coarse basis of M=12
sigma-1 Gaussians with centers spanning [grid_min-0.5, grid_max+0.5].
The projection T (ridge least-squares fit on the empirical x distribution,
done per-call on host) folds into the weights: c[m,i,d] = sum_g T[m,g]
gp_w[i,g,d]. Validated offline: rel err 0.0025 vs 0.0027 for the exact-
basis bf16 path, because |c| stays O(1). Weight DMA drops 17 MB -> 3.4 MB
per core; the kernel becomes output-store-bound (~67 MB bf16 out).

Layout per core (batch shard BL=256), per tile of 4 features, 64 tiles:
  - fb16 quad tile (128, 2048) covers 4 tiles; per tile a (128, 512) slice:
    strip u (u=0,1; PE operand bases must be 0/32/64) rows 64u..64u+11 =
    coarse RBF feats, row 64u+12 = silu; cols = feature (f+u)'s 256 batch
    ++ feature (f+u+2)'s 256 batch. Pad rows hold exp(-30)~1e-13 junk.
  - RBF args via K=5 broadcast matmuls (double-bf16 split of -x^2/2 and x)
    with 2 zero-padded stationaries (5,128) accumulating into ONE (128,512)
    PSUM bank; ONE exp ACT pass per tile (bias -h^2/2 per partition, -30 on
    pad rows). ACT cost is free-size-bound, so strip-stacking halves the
    exp instruction count vs (64,512)-tile v1.
  - silu rows: ONE batched DMA per quad ((2,2048), partitions {12,76}) on
    the ACT HWDGE ring right after the quad's last exp -- same-engine
    program order gives the WAW-after-exp ordering for free, and batching
    keeps the ~0.6us/dma engine-issue cost off the ACT budget.
  - main matmuls MUST be full-height (128,128) stationaries: fast weight
    load only engages for 128-row stationaries (measured 375 vs 730 ns per
    N=512 matmul). Stationary = fb16[0:128, feature cols]; the strip
    structure is compensated in the moving operand: strip-0 features'
    weights live in wtA tiles (rows 0..12 = data, rows 13..127 zeroed
    once), strip-1 features' in wtB tiles (rows 64..76 = data, rest zeroed
    once). Zero regions never overlap the per-group load bands, so weight
    loads never wait on the one-time gpsimd memsets, and fb16 pad junk
    (~1e-13) always multiplies exact zeros.
  - PSUM -> SBUF copies (only ACT and DVE read PSUM) drain 2-bank
    128x1024 tiles, downcast to bf16, balanced so ACT(exp+silu+copies) ~=
    DVE(copies); output DMAs are bf16, issued per (tile, batch-half);
    c=0 stores on the sync HWDGE ring, c=1 stores on gpsimd SWDGE
    (latency-tolerant). Host upcasts bf16 -> f32.
"""

import numpy as np

B, NF, G, D = 2048, 256, 64, 512
NCORES = 8
BL = B // NCORES          # 256 batch rows per core
M = 12                    # coarse Gaussian basis size
KW = M + 1                # weight rows: M coarse + base row
NT = NF // 4              # 64 tiles of 4 features
KF = 5                    # feature-matmul contraction rows

# ACT copies out of 256 total so ACT(exp+silu_dma+copies) ~ DVE(copies)
_NACT = 104

_cache = {}


def _build():
    import concourse.bass as bass
    from concourse import mybir
    from concourse import tile

    f32 = mybir.dt.float32
    bf16 = mybir.dt.bfloat16
    AF = mybir.ActivationFunctionType

    nc = bass.Bass()
    x10 = nc.declare_dram_parameter("x10", [2 * KF, NF * BL // 2], bf16, isOutput=False)
    xsilu = nc.declare_dram_parameter("xsilu", [1, NF * BL], bf16, isOutput=False)
    wcatT = nc.declare_dram_parameter("wcatT", [KW, NF * D], bf16, isOutput=False)
    s10 = nc.declare_dram_parameter("s10", [2 * KF, 128], bf16, isOutput=False)
    biasv = nc.declare_dram_parameter("biasv", [128, 1], f32, isOutput=False)
    zpad = nc.declare_dram_parameter("zpad", [128 - KW, 4 * D], bf16, isOutput=False)
    out = nc.declare_dram_parameter("out", [BL, NF, D], bf16, isOutput=True)

    with tile.TileContext(nc) as tc:
        with (
            tc.tile_pool(name="const", bufs=1) as constp,
            tc.tile_pool(name="x10p", bufs=4) as x10p,
            tc.tile_pool(name="fbqp", bufs=4) as fbqp,
            tc.tile_pool(name="stage", bufs=8) as stagep,
            tc.tile_pool(name="pt", bufs=2, space="PSUM") as ptp,
            tc.tile_pool(name="po", bufs=3, space="PSUM") as pop,
        ):
            s10_t = constp.tile([2 * KF, 128], bf16)
            nc.sync.dma_start(out=s10_t[:, :], in_=s10[:, :])
            biasv_t = constp.tile([128, 1], f32)
            nc.sync.dma_start(out=biasv_t[:, :], in_=biasv[:, :])

            # Persistent weight slots: full-height moving operands with
            # one-time-zeroed pad rows. Strip-0 features in A (data rows
            # 0..KW-1), strip-1 in B (data rows 64..64+KW-1). All pad rows
            # are zero-filled by DMAs from a DRAM zeros block on the gpsimd
            # ring: the transfers ride the startup DMA lull (before stores
            # exist), keep the sync ring free for the critical x10/weight
            # loads, and keep DVE/gpsimd engines clean of memsets. Zero
            # regions never overlap the load bands, so loads don't wait.
            wtA_slots = [
                constp.tile([128, 4 * D], bf16, name=f"wtA{i}") for i in range(4)
            ]
            wtB_slots = [
                constp.tile([128, 4 * D], bf16, name=f"wtB{i}") for i in range(4)
            ]
            for i in range(4):
                nc.gpsimd.dma_start(
                    out=wtA_slots[i][KW:128, :], in_=zpad[0:128 - KW, :]
                )
                nc.gpsimd.dma_start(
                    out=wtB_slots[i][0:64, :], in_=zpad[0:64, :]
                )
                nc.gpsimd.dma_start(
                    out=wtB_slots[i][64 + KW:128, :], in_=zpad[0:64 - KW, :]
                )

            # x10 loads cover 4 tiles each (10, 2048); the silu rows go
            # straight DRAM -> fb16 and are never staged
            def issue_x10_load(q):
                if q >= NT // 4:
                    return None
                x10_t = x10p.tile([2 * KF, 2048], bf16)
                nc.sync.dma_start(
                    out=x10_t[:, :], in_=x10[0:2 * KF, q * 2048:(q + 1) * 2048]
                )
                return x10_t

            # weight loads for group g (2 tiles, 8 features): wcatT is
            # host-permuted so band A = [f,f+2,f+4,f+6], band B =
            # [f+1,f+3,f+5,f+7] are contiguous 4D-column spans
            def issue_wt_load(g):
                if g >= NT // 2:
                    return None
                wtA, wtB = wtA_slots[g % 4], wtB_slots[g % 4]
                nc.sync.dma_start(
                    out=wtA[0:KW, :],
                    in_=wcatT[0:KW, (g * 8) * D:(g * 8 + 4) * D],
                )
                nc.sync.dma_start(
                    out=wtB[64:64 + KW, :],
                    in_=wcatT[0:KW, (g * 8 + 4) * D:(g * 8 + 8) * D],
                )
                return wtA, wtB

            # feature stage for tile t: ONE K=10 matmul (both strips' args
            # via a double-height stationary) -> 1 exp into the quad tile;
            # after the quad's last exp, ONE batched silu DMA
            def feature_stage(t, x10_tiles, fbqs):
                if t >= NT:
                    return
                q, tl = t // 4, t % 4
                if tl == 0:
                    fbqs[q] = fbqp.tile([128, 2048], bf16, name=f"fbq{q}", tag="fbq")
                fbq = fbqs[q]
                x10_t = x10_tiles[q]
                pt = ptp.tile([128, 512], f32)
                nc.tensor.matmul(
                    pt[:, :],
                    s10_t[0:2 * KF, 0:128],
                    x10_t[0:2 * KF, tl * 512:(tl + 1) * 512],
                    start=True,
                    stop=True,
                )
                nc.scalar.activation(
                    fbq[:, tl * 512:(tl + 1) * 512], pt[:, :], AF.Exp,
                    bias=biasv_t[:, :], scale=1.0,
                )
                if tl == 3:
                    silu_dst = fbq[:, :].rearrange(
                        "(u p) (t c) -> u p t c", u=2, t=4
                    )[:, 12, :, :]
                    silu_src = xsilu[0:1, q * 4096:(q + 1) * 4096].rearrange(
                        "one (t u c) -> (one u) t c", t=4, u=2
                    )
                    nc.scalar.dma_start(out=silu_dst, in_=silu_src)

            x10_tiles = {q: issue_x10_load(q) for q in range(3)}
            wts = {0: issue_wt_load(0), 1: issue_wt_load(1)}
            fbqs = {}
            for t0 in range(8):
                feature_stage(t0, x10_tiles, fbqs)

            ncopy = 0
            nact_done = 0
            for t in range(NT):
                if t % 4 == 0:
                    x10_tiles[t // 4 + 3] = issue_x10_load(t // 4 + 3)
                if t % 2 == 0:
                    wts[t // 2 + 2] = issue_wt_load(t // 2 + 2)
                q, tl = t // 4, t % 4
                fbq = fbqs[q]
                if tl == 3:
                    fbqs.pop(q)
                wtA, wtB = wts[t // 2]
                if t % 2 == 1:
                    wts.pop(t // 2)
                iw = t * 4
                st0 = stagep.tile([128, 4 * D], bf16, tag="stage")
                st1 = stagep.tile([128, 4 * D], bf16, tag="stage")
                sts = (st0, st1)
                for j2 in range(2):
                    for c in range(2):
                        po = pop.tile([128, 2 * D], f32)
                        for h in range(2):
                            j = 2 * j2 + h
                            u, cb = j % 2, (j // 2) * 256
                            wt = wtA if u == 0 else wtB
                            nc.tensor.matmul(
                                po[:, h * D:(h + 1) * D],
                                fbq[0:128,
                                    tl * 512 + cb + c * 128:
                                    tl * 512 + cb + c * 128 + 128],
                                wt[0:128,
                                   ((t % 2) * 2 + j // 2) * D:
                                   ((t % 2) * 2 + j // 2 + 1) * D],
                                start=True,
                                stop=True,
                            )
                        dst = sts[c][:, 2 * j2 * D:(2 * j2 + 2) * D]
                        # keep ACT(exp+silu+copies) ~= DVE(copies)
                        use_act = ((ncopy + 1) * _NACT) // 256 > nact_done
                        ncopy += 1
                        if use_act:
                            nact_done += 1
                            nc.scalar.copy(dst, po[:, :])
                        else:
                            nc.vector.tensor_scalar_mul(dst, po[:, :], 1.0)
                        if j2 == 1:
                            eng_dma = nc.sync if c == 0 else nc.gpsimd
                            eng_dma.dma_start(
                                out=out[c * 128:(c + 1) * 128, iw:iw + 4, :],
                                in_=sts[c][:, :],
                            )
                # staged AFTER the tile's copies: the exp would otherwise
                # sit ahead of them in ACT's strict-FIFO queue and, when
                # the PE lags, head-of-line-block the copies -> po backs
                # up -> PE stalls more -> HAM goes cold (vicious cycle)
                feature_stage(t + 8, x10_tiles, fbqs)

    _split_multi_waits(nc)
    return nc


def _split_multi_waits(nc):
    """Walrus TPB instruction structs accept a single sync wait. Hoist all
    but the last wait of any instruction onto same-engine NOPs inserted
    immediately before it (a wait executes before the instruction either
    way, so this is semantically identical)."""
    import dataclasses
    import concourse.bass as bass
    import concourse.mybir as mybir

    tpl = bass.Bass().sync.nop().ins
    k = 0
    for blk in nc.m.functions[0].blocks:
        out_insts = []
        for inst in blk.instructions:
            si = getattr(inst, "sync_info", None)
            if si is not None and len(si.on_wait) > 1:
                for w in si.on_wait[:-1]:
                    out_insts.append(
                        dataclasses.replace(
                            tpl,
                            name=f"nop-w{k}",
                            engine=inst.engine,
                            sync_info=mybir.SyncInfo(on_wait=[w], on_update=[]),
                        )
                    )
                    k += 1
                inst.sync_info = dataclasses.replace(si, on_wait=si.on_wait[-1:])
            out_insts.append(inst)
        blk.instructions[:] = out_insts


def _hi_lo(v, bf16):
    """Double-bf16 split: v ~= hi + lo with |err| <~ |v| * 2^-17."""
    hi = v.astype(bf16)
    lo = (v - hi.astype(np.float32)).astype(bf16)
    return hi, lo


def _fit_basis(x, grid):
    """Ridge-fit the projection T (M x G) of the grid Gaussians onto M
    coarse sigma-1 Gaussians, weighted by the empirical x distribution."""
    xf = x.ravel().astype(np.float64)
    xs_emp = xf[::17][:30000]
    lo, hi = xf.min() - 0.3, xf.max() + 0.3
    xs_uni = np.linspace(lo, hi, 4000)
    xs = np.concatenate([xs_emp, xs_uni])
    w_s = np.concatenate(
        [np.full(xs_emp.size, 1.0), np.full(xs_uni.size, 0.05)]
    )
    g64 = grid.astype(np.float64)
    ctr = np.linspace(g64.min() - 0.5, g64.max() + 0.5, M)
    PHI = np.exp(-0.5 * (xs[:, None] - g64[None, :]) ** 2)
    PSI = np.exp(-0.5 * (xs[:, None] - ctr[None, :]) ** 2)
    Aw = PSI * np.sqrt(w_s)[:, None]
    Bw = PHI * np.sqrt(w_s)[:, None]
    T = np.linalg.solve(Aw.T @ Aw + 1e-4 * np.eye(M), Aw.T @ Bw)
    return ctr, T


def _prep_inputs(x, base_weight, gp_weight, grid):
    import ml_dtypes

    bf16 = ml_dtypes.bfloat16
    x = np.ascontiguousarray(np.asarray(x, np.float32))
    base_weight = np.asarray(base_weight, np.float32)
    gp_weight = np.asarray(gp_weight, np.float32)
    grid = np.asarray(grid, np.float32)

    ctr, T = _fit_basis(x, grid)

    # x6 feature order: per tile of 4, strip u holds (f+u, f+u+2)
    permx = np.empty(NF, np.int64)
    k = 0
    for t in range(NT):
        for u in range(2):
            permx[k] = t * 4 + u
            permx[k + 1] = t * 4 + u + 2
            k += 2

    # wcatT feature order: per group of 8 (2 tiles), band A holds
    # [f, f+2, f+4, f+6], band B [f+1, f+3, f+5, f+7], each contiguous
    permw = np.empty(NF, np.int64)
    k = 0
    for g in range(NT // 2):
        f = g * 8
        for u in range(2):
            for v in (0, 2, 4, 6):
                permw[k] = f + u + v
                k += 1

    # c[m, i, d] = sum_g T[m, g] gp_w[i, g, d]; row M = base_weight
    cw = np.einsum("mg,igd->mid", T, gp_weight.astype(np.float64))
    wcat = np.concatenate([cw, base_weight[None].astype(np.float64)], axis=0)
    wcatT = np.ascontiguousarray(
        wcat[:, permw, :].reshape(KW, NF * D).astype(bf16)
    )

    h_hi, h_lo = _hi_lo(ctr.astype(np.float32), bf16)
    # double-height stationary (10, 128): rows 0..4 pair with strip-0
    # moving rows and activate cols 0..M-1 (PSUM partitions 0..M-1);
    # rows 5..9 pair with strip-1 moving rows, cols 64..64+M-1
    s10 = np.zeros((2 * KF, 128), bf16)
    for u in range(2):
        o = 64 * u
        r = KF * u
        s10[r + 0, o:o + M] = np.ones(M, bf16)
        s10[r + 1, o:o + M] = np.ones(M, bf16)
        s10[r + 2, o:o + M] = h_hi
        s10[r + 3, o:o + M] = h_lo
        s10[r + 4, o:o + M] = h_hi
    s10 = np.ascontiguousarray(s10)

    biasv = np.full((128, 1), -30.0, np.float32)
    for u in range(2):
        biasv[64 * u:64 * u + M, 0] = (-0.5 * ctr * ctr).astype(np.float32)
    biasv = np.ascontiguousarray(biasv)

    # even/odd feature splits per tile: strip 0 moving = [f, f+2],
    # strip 1 moving = [f+1, f+3]
    perm_ev = np.empty(NF // 2, np.int64)
    perm_od = np.empty(NF // 2, np.int64)
    k = 0
    for t in range(NT):
        perm_ev[k] = t * 4
        perm_ev[k + 1] = t * 4 + 2
        perm_od[k] = t * 4 + 1
        perm_od[k + 1] = t * 4 + 3
        k += 2

    def _x5_rows(xTs):
        mx2 = (-0.5 * xTs * xTs).ravel()
        xr = xTs.ravel()
        mx2_hi, mx2_lo = _hi_lo(mx2, bf16)
        x_hi, x_lo = _hi_lo(xr, bf16)
        return [mx2_hi, mx2_lo, x_hi, x_hi, x_lo]

    in_maps = []
    for cidx in range(NCORES):
        xT = np.ascontiguousarray(x[cidx * BL:(cidx + 1) * BL, :].T)  # (NF, BL)
        xTp = xT[permx]
        x10 = np.ascontiguousarray(
            np.stack(_x5_rows(xT[perm_ev]) + _x5_rows(xT[perm_od]))
        )
        xsilu = np.ascontiguousarray(
            (xTp / (1.0 + np.exp(-xTp))).ravel().astype(bf16).reshape(1, -1)
        )
        in_maps.append(
            {"x10": x10, "xsilu": xsilu, "wcatT": wcatT, "s10": s10,
             "biasv": biasv, "zpad": np.zeros((128 - KW, 4 * D), bf16)}
        )
    return in_maps


def _run(in_maps, **kw):
    from concourse.bass_utils import run_bass_kernel_spmd

    if "nc" not in _cache:
        _cache["nc"] = _build()
    return run_bass_kernel_spmd(_cache["nc"], in_maps, list(range(NCORES)), **kw)


def kernel(x, base_weight, gp_weight, grid):
    in_maps = _prep_inputs(x, base_weight, gp_weight, grid)
    res = _run(in_maps)
    return np.concatenate(
        [np.asarray(r["out"]).astype(np.float32) for r in res.results], axis=0
    )


# revision 34
# speedup vs baseline: 1.3192x; 1.0482x over previous
"""KAN embeddings Bass kernel for Trainium2, 8-core data-parallel over batch.

out[b,i,d] = silu(x[b,i]) * base_w[i,d] + sum_g exp(-0.5(x[b,i]-grid[g])^2) * gp_w[i,g,d]

v2.1 strategy: the 64 grid Gaussians (sigma=1, spacing 0.063) are massively
oversampled -- the gp branch is re-expressed in a coarse basis of M=12
sigma-1 Gaussians with centers spanning [grid_min-0.5, grid_max+0.5].
The projection T (ridge least-squares fit on the empirical x distribution,
done per-call on host) folds into the weights: c[m,i,d] = sum_g T[m,g]
gp_w[i,g,d]. Validated offline: rel err 0.0025 vs 0.0027 for the exact-
basis bf16 path, because |c| stays O(1). Weight DMA drops 17 MB -> 3.4 MB
per core; the kernel becomes output-store-bound (~67 MB bf16 out).

Layout per core (batch shard BL=256), per tile of 4 features, 64 tiles:
  - fb16 quad tile (128, 2048) covers 4 tiles; per tile a (128, 512) slice:
    strip u (u=0,1; PE operand bases must be 0/32/64) rows 64u..64u+11 =
    coarse RBF feats, row 64u+12 = silu; cols = feature (f+u)'s 256 batch
    ++ feature (f+u+2)'s 256 batch. Pad rows hold exp(-30)~1e-13 junk.
  - RBF args via K=5 broadcast matmuls (double-bf16 split of -x^2/2 and x)
    with 2 zero-padded stationaries (5,128) accumulating into ONE (128,512)
    PSUM bank; ONE exp ACT pass per tile (bias -h^2/2 per partition, -30 on
    pad rows). ACT cost is free-size-bound, so strip-stacking halves the
    exp instruction count vs (64,512)-tile v1.
  - silu rows: ONE batched DMA per quad ((2,2048), partitions {12,76}) on
    the ACT HWDGE ring right after the quad's last exp -- same-engine
    program order gives the WAW-after-exp ordering for free, and batching
    keeps the ~0.6us/dma engine-issue cost off the ACT budget.
  - main matmuls MUST be full-height (128,128) stationaries: fast weight
    load only engages for 128-row stationaries (measured 375 vs 730 ns per
    N=512 matmul). Stationary = fb16[0:128, feature cols]; the strip
    structure is compensated in the moving operand: strip-0 features'
    weights live in wtA tiles (rows 0..12 = data, rows 13..127 zeroed
    once), strip-1 features' in wtB tiles (rows 64..76 = data, rest zeroed
    once). Zero regions never overlap the per-group load bands, so weight
    loads never wait on the one-time gpsimd memsets, and fb16 pad junk
    (~1e-13) always multiplies exact zeros.
  - PSUM -> SBUF copies (only ACT and DVE read PSUM) drain 2-bank
    128x1024 tiles, downcast to bf16, balanced so ACT(exp+silu+copies) ~=
    DVE(copies); output DMAs are bf16, issued per (tile, batch-half);
    c=0 stores on the sync HWDGE ring, c=1 stores on gpsimd SWDGE
    (latency-tolerant). Host upcasts bf16 -> f32.
"""

import numpy as np

B, NF, G, D = 2048, 256, 64, 512
NCORES = 8
BL = B // NCORES          # 256 batch rows per core
M = 12                    # coarse Gaussian basis size
KW = M + 1                # weight rows: M coarse + base row
NT = NF // 4              # 64 tiles of 4 features
KF = 5                    # feature-matmul contraction rows

# ACT copies out of 256 total so ACT(exp+silu_dma+copies) ~ DVE(copies)
_NACT = 104

_cache = {}


def _build():
    import concourse.bass as bass
    from concourse import mybir
    from concourse import tile

    f32 = mybir.dt.float32
    bf16 = mybir.dt.bfloat16
    AF = mybir.ActivationFunctionType

    nc = bass.Bass()
    x10 = nc.declare_dram_parameter("x10", [2 * KF, NF * BL // 2], bf16, isOutput=False)
    xsilu = nc.declare_dram_parameter("xsilu", [1, NF * BL], bf16, isOutput=False)
    wcatT = nc.declare_dram_parameter("wcatT", [KW, NF * D], bf16, isOutput=False)
    s10 = nc.declare_dram_parameter("s10", [2 * KF, 128], bf16, isOutput=False)
    biasv = nc.declare_dram_parameter("biasv", [128, 1], f32, isOutput=False)
    zpad = nc.declare_dram_parameter("zpad", [32 - KW, 4 * D], bf16, isOutput=False)
    fbq0 = nc.declare_dram_parameter("fbq0", [128, 2048], bf16, isOutput=False)
    out = nc.declare_dram_parameter("out", [BL, NF, D], bf16, isOutput=True)

    with tile.TileContext(nc) as tc:
        with (
            tc.tile_pool(name="const", bufs=1) as constp,
            tc.tile_pool(name="x10p", bufs=4) as x10p,
            tc.tile_pool(name="fbqp", bufs=4) as fbqp,
            tc.tile_pool(name="stage", bufs=8) as stagep,
            tc.tile_pool(name="pt", bufs=2, space="PSUM") as ptp,
            tc.tile_pool(name="po", bufs=3, space="PSUM") as pop,
        ):
            s10_t = constp.tile([2 * KF, 128], bf16)
            nc.sync.dma_start(out=s10_t[:, :], in_=s10[:, :])
            biasv_t = constp.tile([128, 1], f32)
            nc.sync.dma_start(out=biasv_t[:, :], in_=biasv[:, :])

            # Quad 0's feature tile is host-precomputed and DMA'd whole:
            # the first mains then skip the entire feature-pipeline warmup
            # (4 matmuls + 4 exps + silu) at startup.
            fbq0_t = constp.tile([128, 2048], bf16)
            nc.sync.dma_start(out=fbq0_t[:, :], in_=fbq0[:, :])

            # Persistent weight slots (3 suffice: groups g..g+2 in flight):
            # full-height moving operands with one-time-zeroed pad rows.
            # Strip-0 features in A (data rows 0..KW-1), strip-1 in B (data
            # rows 64..64+KW-1). Bulk zeros are 32-aligned memsets -- A on
            # vector, B on gpsimd, slot 0 first, so slot i is ready just
            # ahead of its group's first main matmul; the unaligned 19-row
            # slivers come from a DRAM zeros block on the gpsimd ring.
            # Zero regions never overlap the load bands, so loads never
            # wait on the zeroing.
            wtA_slots = [
                constp.tile([128, 4 * D], bf16, name=f"wtA{i}") for i in range(3)
            ]
            wtB_slots = [
                constp.tile([128, 4 * D], bf16, name=f"wtB{i}") for i in range(3)
            ]
            for i in range(3):
                nc.gpsimd.dma_start(out=wtA_slots[i][KW:32, :], in_=zpad[:, :])
                nc.gpsimd.dma_start(
                    out=wtB_slots[i][64 + KW:96, :], in_=zpad[:, :]
                )
                nc.vector.memset(wtA_slots[i][32:64, :], 0.0)
                nc.vector.memset(wtA_slots[i][64:128, :], 0.0)
                nc.gpsimd.memset(wtB_slots[i][0:64, :], 0.0)
                nc.gpsimd.memset(wtB_slots[i][96:128, :], 0.0)

            # x10 loads cover 4 tiles each (10, 2048); the silu rows go
            # straight DRAM -> fb16 and are never staged
            def issue_x10_load(q):
                if q >= NT // 4:
                    return None
                x10_t = x10p.tile([2 * KF, 2048], bf16)
                nc.sync.dma_start(
                    out=x10_t[:, :], in_=x10[0:2 * KF, q * 2048:(q + 1) * 2048]
                )
                return x10_t

            # weight loads for group g (2 tiles, 8 features): wcatT is
            # host-permuted so band A = [f,f+2,f+4,f+6], band B =
            # [f+1,f+3,f+5,f+7] are contiguous 4D-column spans
            def issue_wt_load(g):
                if g >= NT // 2:
                    return None
                wtA, wtB = wtA_slots[g % 3], wtB_slots[g % 3]
                nc.sync.dma_start(
                    out=wtA[0:KW, :],
                    in_=wcatT[0:KW, (g * 8) * D:(g * 8 + 4) * D],
                )
                nc.sync.dma_start(
                    out=wtB[64:64 + KW, :],
                    in_=wcatT[0:KW, (g * 8 + 4) * D:(g * 8 + 8) * D],
                )
                return wtA, wtB

            # feature stage for tile t: ONE K=10 matmul (both strips' args
            # via a double-height stationary) -> 1 exp into the quad tile;
            # after the quad's last exp, ONE batched silu DMA
            def feature_stage(t, x10_tiles, fbqs):
                if t >= NT:
                    return
                q, tl = t // 4, t % 4
                if tl == 0:
                    fbqs[q] = fbqp.tile([128, 2048], bf16, name=f"fbq{q}", tag="fbq")
                fbq = fbqs[q]
                x10_t = x10_tiles[q]
                pt = ptp.tile([128, 512], f32)
                nc.tensor.matmul(
                    pt[:, :],
                    s10_t[0:2 * KF, 0:128],
                    x10_t[0:2 * KF, tl * 512:(tl + 1) * 512],
                    start=True,
                    stop=True,
                )
                nc.scalar.activation(
                    fbq[:, tl * 512:(tl + 1) * 512], pt[:, :], AF.Exp,
                    bias=biasv_t[:, :], scale=1.0,
                )
                if tl == 3:
                    silu_dst = fbq[:, :].rearrange(
                        "(u p) (t c) -> u p t c", u=2, t=4
                    )[:, 12, :, :]
                    silu_src = xsilu[0:1, q * 4096:(q + 1) * 4096].rearrange(
                        "one (t u c) -> (one u) t c", t=4, u=2
                    )
                    nc.scalar.dma_start(out=silu_dst, in_=silu_src)

            x10_tiles = {q: issue_x10_load(q) for q in range(3)}
            wts = {0: issue_wt_load(0), 1: issue_wt_load(1)}
            fbqs = {0: fbq0_t}
            for t0 in range(4, 8):
                feature_stage(t0, x10_tiles, fbqs)

            ncopy = 0
            nact_done = 0
            for t in range(NT):
                if t % 4 == 0:
                    x10_tiles[t // 4 + 3] = issue_x10_load(t // 4 + 3)
                if t % 2 == 0:
                    wts[t // 2 + 2] = issue_wt_load(t // 2 + 2)
                q, tl = t // 4, t % 4
                fbq = fbqs[q]
                if tl == 3:
                    fbqs.pop(q)
                wtA, wtB = wts[t // 2]
                if t % 2 == 1:
                    wts.pop(t // 2)
                iw = t * 4
                st0 = stagep.tile([128, 4 * D], bf16, tag="stage")
                st1 = stagep.tile([128, 4 * D], bf16, tag="stage")
                sts = (st0, st1)
                for j2 in range(2):
                    for c in range(2):
                        po = pop.tile([128, 2 * D], f32)
                        for h in range(2):
                            j = 2 * j2 + h
                            u, cb = j % 2, (j // 2) * 256
                            wt = wtA if u == 0 else wtB
                            nc.tensor.matmul(
                                po[:, h * D:(h + 1) * D],
                                fbq[0:128,
                                    tl * 512 + cb + c * 128:
                                    tl * 512 + cb + c * 128 + 128],
                                wt[0:128,
                                   ((t % 2) * 2 + j // 2) * D:
                                   ((t % 2) * 2 + j // 2 + 1) * D],
                                start=True,
                                stop=True,
                            )
                        dst = sts[c][:, 2 * j2 * D:(2 * j2 + 2) * D]
                        # keep ACT(exp+silu+copies) ~= DVE(copies)
                        use_act = ((ncopy + 1) * _NACT) // 256 > nact_done
                        ncopy += 1
                        if use_act:
                            nact_done += 1
                            nc.scalar.copy(dst, po[:, :])
                        else:
                            nc.vector.tensor_scalar_mul(dst, po[:, :], 1.0)
                        if j2 == 1:
                            # c=1 stores ride SWDGE (latency-tolerant)
                            # except near the end, where gpsimd's slow
                            # descriptor path would become the tail drain
                            eng_dma = (
                                nc.sync if (c == 0 or t >= NT - 4) else nc.gpsimd
                            )
                            eng_dma.dma_start(
                                out=out[c * 128:(c + 1) * 128, iw:iw + 4, :],
                                in_=sts[c][:, :],
                            )
                # staged AFTER the tile's copies: the exp would otherwise
                # sit ahead of them in ACT's strict-FIFO queue and, when
                # the PE lags, head-of-line-block the copies -> po backs
                # up -> PE stalls more -> HAM goes cold (vicious cycle)
                feature_stage(t + 8, x10_tiles, fbqs)

    _split_multi_waits(nc)
    return nc


def _split_multi_waits(nc):
    """Walrus TPB instruction structs accept a single sync wait. Hoist all
    but the last wait of any instruction onto same-engine NOPs inserted
    immediately before it (a wait executes before the instruction either
    way, so this is semantically identical)."""
    import dataclasses
    import concourse.bass as bass
    import concourse.mybir as mybir

    tpl = bass.Bass().sync.nop().ins
    k = 0
    for blk in nc.m.functions[0].blocks:
        out_insts = []
        for inst in blk.instructions:
            si = getattr(inst, "sync_info", None)
            if si is not None and len(si.on_wait) > 1:
                for w in si.on_wait[:-1]:
                    out_insts.append(
                        dataclasses.replace(
                            tpl,
                            name=f"nop-w{k}",
                            engine=inst.engine,
                            sync_info=mybir.SyncInfo(on_wait=[w], on_update=[]),
                        )
                    )
                    k += 1
                inst.sync_info = dataclasses.replace(si, on_wait=si.on_wait[-1:])
            out_insts.append(inst)
        blk.instructions[:] = out_insts


def _hi_lo(v, bf16):
    """Double-bf16 split: v ~= hi + lo with |err| <~ |v| * 2^-17."""
    hi = v.astype(bf16)
    lo = (v - hi.astype(np.float32)).astype(bf16)
    return hi, lo


def _fit_basis(x, grid):
    """Ridge-fit the projection T (M x G) of the grid Gaussians onto M
    coarse sigma-1 Gaussians, weighted by the empirical x distribution."""
    xf = x.ravel().astype(np.float64)
    xs_emp = xf[::17][:30000]
    lo, hi = xf.min() - 0.3, xf.max() + 0.3
    xs_uni = np.linspace(lo, hi, 4000)
    xs = np.concatenate([xs_emp, xs_uni])
    w_s = np.concatenate(
        [np.full(xs_emp.size, 1.0), np.full(xs_uni.size, 0.05)]
    )
    g64 = grid.astype(np.float64)
    ctr = np.linspace(g64.min() - 0.5, g64.max() + 0.5, M)
    PHI = np.exp(-0.5 * (xs[:, None] - g64[None, :]) ** 2)
    PSI = np.exp(-0.5 * (xs[:, None] - ctr[None, :]) ** 2)
    Aw = PSI * np.sqrt(w_s)[:, None]
    Bw = PHI * np.sqrt(w_s)[:, None]
    T = np.linalg.solve(Aw.T @ Aw + 1e-4 * np.eye(M), Aw.T @ Bw)
    return ctr, T


def _prep_inputs(x, base_weight, gp_weight, grid):
    import ml_dtypes

    bf16 = ml_dtypes.bfloat16
    x = np.ascontiguousarray(np.asarray(x, np.float32))
    base_weight = np.asarray(base_weight, np.float32)
    gp_weight = np.asarray(gp_weight, np.float32)
    grid = np.asarray(grid, np.float32)

    ctr, T = _fit_basis(x, grid)

    # x6 feature order: per tile of 4, strip u holds (f+u, f+u+2)
    permx = np.empty(NF, np.int64)
    k = 0
    for t in range(NT):
        for u in range(2):
            permx[k] = t * 4 + u
            permx[k + 1] = t * 4 + u + 2
            k += 2

    # wcatT feature order: per group of 8 (2 tiles), band A holds
    # [f, f+2, f+4, f+6], band B [f+1, f+3, f+5, f+7], each contiguous
    permw = np.empty(NF, np.int64)
    k = 0
    for g in range(NT // 2):
        f = g * 8
        for u in range(2):
            for v in (0, 2, 4, 6):
                permw[k] = f + u + v
                k += 1

    # c[m, i, d] = sum_g T[m, g] gp_w[i, g, d]; row M = base_weight
    cw = np.einsum("mg,igd->mid", T, gp_weight.astype(np.float64))
    wcat = np.concatenate([cw, base_weight[None].astype(np.float64)], axis=0)
    wcatT = np.ascontiguousarray(
        wcat[:, permw, :].reshape(KW, NF * D).astype(bf16)
    )

    h_hi, h_lo = _hi_lo(ctr.astype(np.float32), bf16)
    # double-height stationary (10, 128): rows 0..4 pair with strip-0
    # moving rows and activate cols 0..M-1 (PSUM partitions 0..M-1);
    # rows 5..9 pair with strip-1 moving rows, cols 64..64+M-1
    s10 = np.zeros((2 * KF, 128), bf16)
    for u in range(2):
        o = 64 * u
        r = KF * u
        s10[r + 0, o:o + M] = np.ones(M, bf16)
        s10[r + 1, o:o + M] = np.ones(M, bf16)
        s10[r + 2, o:o + M] = h_hi
        s10[r + 3, o:o + M] = h_lo
        s10[r + 4, o:o + M] = h_hi
    s10 = np.ascontiguousarray(s10)

    biasv = np.full((128, 1), -30.0, np.float32)
    for u in range(2):
        biasv[64 * u:64 * u + M, 0] = (-0.5 * ctr * ctr).astype(np.float32)
    biasv = np.ascontiguousarray(biasv)

    # even/odd feature splits per tile: strip 0 moving = [f, f+2],
    # strip 1 moving = [f+1, f+3]
    perm_ev = np.empty(NF // 2, np.int64)
    perm_od = np.empty(NF // 2, np.int64)
    k = 0
    for t in range(NT):
        perm_ev[k] = t * 4
        perm_ev[k + 1] = t * 4 + 2
        perm_od[k] = t * 4 + 1
        perm_od[k + 1] = t * 4 + 3
        k += 2

    def _x5_rows(xTs):
        mx2 = (-0.5 * xTs * xTs).ravel()
        xr = xTs.ravel()
        mx2_hi, mx2_lo = _hi_lo(mx2, bf16)
        x_hi, x_lo = _hi_lo(xr, bf16)
        return [mx2_hi, mx2_lo, x_hi, x_hi, x_lo]

    in_maps = []
    for cidx in range(NCORES):
        xT = np.ascontiguousarray(x[cidx * BL:(cidx + 1) * BL, :].T)  # (NF, BL)
        xTp = xT[permx]
        x10 = np.ascontiguousarray(
            np.stack(_x5_rows(xT[perm_ev]) + _x5_rows(xT[perm_od]))
        )
        xsilu = np.ascontiguousarray(
            (xTp / (1.0 + np.exp(-xTp))).ravel().astype(bf16).reshape(1, -1)
        )
        # host-precomputed quad-0 feature tile (mirrors the device math:
        # bf16 operands, f32 args, exp, bf16 store, silu rows overwritten)
        x10f = x10.astype(np.float32)
        fbq0 = np.empty((128, 2048), bf16)
        for tl in range(4):
            pt = s10.astype(np.float32).T @ x10f[:, tl * 512:(tl + 1) * 512]
            fbq0[:, tl * 512:(tl + 1) * 512] = np.exp(pt + biasv).astype(bf16)
        ssrc = np.asarray(xsilu[0, 0:4096].reshape(4, 2, 512))
        for u in range(2):
            fbq0[64 * u + 12, :] = ssrc[:, u, :].reshape(4 * 512)
        fbq0 = np.ascontiguousarray(fbq0)
        in_maps.append(
            {"x10": x10, "xsilu": xsilu, "wcatT": wcatT, "s10": s10,
             "biasv": biasv, "zpad": np.zeros((32 - KW, 4 * D), bf16),
             "fbq0": fbq0}
        )
    return in_maps


def _run(in_maps, **kw):
    from concourse.bass_utils import run_bass_kernel_spmd

    if "nc" not in _cache:
        _cache["nc"] = _build()
    return run_bass_kernel_spmd(_cache["nc"], in_maps, list(range(NCORES)), **kw)


def kernel(x, base_weight, gp_weight, grid):
    in_maps = _prep_inputs(x, base_weight, gp_weight, grid)
    res = _run(in_maps)
    return np.concatenate(
        [np.asarray(r["out"]).astype(np.float32) for r in res.results], axis=0
    )


# revision 35
# speedup vs baseline: 1.3541x; 1.0265x over previous
"""KAN embeddings Bass kernel for Trainium2, 8-core data-parallel over batch.

out[b,i,d] = silu(x[b,i]) * base_w[i,d] + sum_g exp(-0.5(x[b,i]-grid[g])^2) * gp_w[i,g,d]

v2.1 strategy: the 64 grid Gaussians (sigma=1, spacing 0.063) are massively
oversampled -- the gp branch is re-expressed in a coarse basis of M=12
sigma-1 Gaussians with centers spanning [grid_min-0.5, grid_max+0.5].
The projection T (ridge least-squares fit on the empirical x distribution,
done per-call on host) folds into the weights: c[m,i,d] = sum_g T[m,g]
gp_w[i,g,d]. Validated offline: rel err 0.0025 vs 0.0027 for the exact-
basis bf16 path, because |c| stays O(1). Weight DMA drops 17 MB -> 3.4 MB
per core; the kernel becomes output-store-bound (~67 MB bf16 out).

Layout per core (batch shard BL=256), per tile of 4 features, 64 tiles:
  - fb16 quad tile (128, 2048) covers 4 tiles; per tile a (128, 512) slice:
    strip u (u=0,1; PE operand bases must be 0/32/64) rows 64u..64u+11 =
    coarse RBF feats, row 64u+12 = silu; cols = feature (f+u)'s 256 batch
    ++ feature (f+u+2)'s 256 batch. Pad rows hold exp(-30)~1e-13 junk.
  - RBF args via K=5 broadcast matmuls (double-bf16 split of -x^2/2 and x)
    with 2 zero-padded stationaries (5,128) accumulating into ONE (128,512)
    PSUM bank; ONE exp ACT pass per tile (bias -h^2/2 per partition, -30 on
    pad rows). ACT cost is free-size-bound, so strip-stacking halves the
    exp instruction count vs (64,512)-tile v1.
  - silu rows: ONE batched DMA per quad ((2,2048), partitions {12,76}) on
    the ACT HWDGE ring right after the quad's last exp -- same-engine
    program order gives the WAW-after-exp ordering for free, and batching
    keeps the ~0.6us/dma engine-issue cost off the ACT budget.
  - main matmuls MUST be full-height (128,128) stationaries: fast weight
    load only engages for 128-row stationaries (measured 375 vs 730 ns per
    N=512 matmul). Stationary = fb16[0:128, feature cols]; the strip
    structure is compensated in the moving operand: strip-0 features'
    weights live in wtA tiles (rows 0..12 = data, rows 13..127 zeroed
    once), strip-1 features' in wtB tiles (rows 64..76 = data, rest zeroed
    once). Zero regions never overlap the per-group load bands, so weight
    loads never wait on the one-time gpsimd memsets, and fb16 pad junk
    (~1e-13) always multiplies exact zeros.
  - PSUM -> SBUF copies (only ACT and DVE read PSUM) drain 2-bank
    128x1024 tiles, downcast to bf16, balanced so ACT(exp+silu+copies) ~=
    DVE(copies); output DMAs are bf16, issued per (tile, batch-half);
    c=0 stores on the sync HWDGE ring, c=1 stores on gpsimd SWDGE
    (latency-tolerant). Host upcasts bf16 -> f32.
"""

import numpy as np

B, NF, G, D = 2048, 256, 64, 512
NCORES = 8
BL = B // NCORES          # 256 batch rows per core
M = 12                    # coarse Gaussian basis size
KW = M + 1                # weight rows: M coarse + base row
NT = NF // 4              # 64 tiles of 4 features
KF = 5                    # feature-matmul contraction rows

# ACT copies out of 256 total so ACT(exp+silu_dma+copies) ~ DVE(copies)
_NACT = 116

_cache = {}


def _build():
    import concourse.bass as bass
    from concourse import mybir
    from concourse import tile

    f32 = mybir.dt.float32
    bf16 = mybir.dt.bfloat16
    AF = mybir.ActivationFunctionType

    nc = bass.Bass()
    x10 = nc.declare_dram_parameter("x10", [2 * KF, NF * BL // 2], bf16, isOutput=False)
    xsilu = nc.declare_dram_parameter("xsilu", [1, NF * BL], bf16, isOutput=False)
    wcatT = nc.declare_dram_parameter("wcatT", [KW, NF * D], bf16, isOutput=False)
    s10 = nc.declare_dram_parameter("s10", [2 * KF, 128], bf16, isOutput=False)
    biasv = nc.declare_dram_parameter("biasv", [128, 1], f32, isOutput=False)
    zpad = nc.declare_dram_parameter("zpad", [32 - KW, 4 * D], bf16, isOutput=False)
    fbq0 = nc.declare_dram_parameter("fbq0", [128, 2048], bf16, isOutput=False)
    out = nc.declare_dram_parameter("out", [BL, NF, D], bf16, isOutput=True)

    with tile.TileContext(nc) as tc:
        with (
            tc.tile_pool(name="const", bufs=1) as constp,
            tc.tile_pool(name="x10p", bufs=4) as x10p,
            tc.tile_pool(name="fbqp", bufs=4) as fbqp,
            tc.tile_pool(name="stage", bufs=10) as stagep,
            tc.tile_pool(name="pt", bufs=2, space="PSUM") as ptp,
            tc.tile_pool(name="po", bufs=3, space="PSUM") as pop,
        ):
            s10_t = constp.tile([2 * KF, 128], bf16)
            nc.sync.dma_start(out=s10_t[:, :], in_=s10[:, :])
            biasv_t = constp.tile([128, 1], f32)
            nc.sync.dma_start(out=biasv_t[:, :], in_=biasv[:, :])

            # Quad 0's feature tile is host-precomputed and DMA'd whole:
            # the first mains then skip the entire feature-pipeline warmup
            # (4 matmuls + 4 exps + silu) at startup.
            fbq0_t = constp.tile([128, 2048], bf16)
            nc.sync.dma_start(out=fbq0_t[:, :], in_=fbq0[:, :])

            # Persistent weight slots (3 suffice: groups g..g+2 in flight):
            # full-height moving operands with one-time-zeroed pad rows.
            # Strip-0 features in A (data rows 0..KW-1), strip-1 in B (data
            # rows 64..64+KW-1). Bulk zeros are 32-aligned memsets -- A on
            # vector, B on gpsimd, slot 0 first, so slot i is ready just
            # ahead of its group's first main matmul; the unaligned 19-row
            # slivers come from a DRAM zeros block on the gpsimd ring.
            # Zero regions never overlap the load bands, so loads never
            # wait on the zeroing.
            wtA_slots = [
                constp.tile([128, 4 * D], bf16, name=f"wtA{i}") for i in range(3)
            ]
            wtB_slots = [
                constp.tile([128, 4 * D], bf16, name=f"wtB{i}") for i in range(3)
            ]
            def _msA(i, eng):
                eng.memset(wtA_slots[i][32:64, :], 0.0)
                eng.memset(wtA_slots[i][64:128, :], 0.0)

            def _msB(i, eng):
                eng.memset(wtB_slots[i][0:64, :], 0.0)
                eng.memset(wtB_slots[i][96:128, :], 0.0)

            # slot 0 (and A1) zeroed on vector before its copies ramp;
            # the rest on gpsimd interleaved with the sliver-fill DMAs,
            # ordered so each slot is ready ahead of its group's mains
            _msA(0, nc.vector)
            _msB(0, nc.vector)
            _msA(1, nc.vector)
            nc.gpsimd.dma_start(out=wtA_slots[0][KW:32, :], in_=zpad[:, :])
            nc.gpsimd.dma_start(out=wtB_slots[0][64 + KW:96, :], in_=zpad[:, :])
            _msB(1, nc.gpsimd)
            nc.gpsimd.dma_start(out=wtA_slots[1][KW:32, :], in_=zpad[:, :])
            nc.gpsimd.dma_start(out=wtB_slots[1][64 + KW:96, :], in_=zpad[:, :])
            _msA(2, nc.gpsimd)
            _msB(2, nc.gpsimd)
            nc.gpsimd.dma_start(out=wtA_slots[2][KW:32, :], in_=zpad[:, :])
            nc.gpsimd.dma_start(out=wtB_slots[2][64 + KW:96, :], in_=zpad[:, :])

            # x10 loads cover 4 tiles each (10, 2048); the silu rows go
            # straight DRAM -> fb16 and are never staged
            def issue_x10_load(q):
                if q >= NT // 4:
                    return None
                x10_t = x10p.tile([2 * KF, 2048], bf16)
                nc.sync.dma_start(
                    out=x10_t[:, :], in_=x10[0:2 * KF, q * 2048:(q + 1) * 2048]
                )
                return x10_t

            # weight loads for group g (2 tiles, 8 features): wcatT is
            # host-permuted so band A = [f,f+2,f+4,f+6], band B =
            # [f+1,f+3,f+5,f+7] are contiguous 4D-column spans
            def issue_wt_load(g):
                if g >= NT // 2:
                    return None
                wtA, wtB = wtA_slots[g % 3], wtB_slots[g % 3]
                nc.sync.dma_start(
                    out=wtA[0:KW, :],
                    in_=wcatT[0:KW, (g * 8) * D:(g * 8 + 4) * D],
                )
                nc.sync.dma_start(
                    out=wtB[64:64 + KW, :],
                    in_=wcatT[0:KW, (g * 8 + 4) * D:(g * 8 + 8) * D],
                )
                return wtA, wtB

            # feature stage for tile t: ONE K=10 matmul (both strips' args
            # via a double-height stationary) -> 1 exp into the quad tile;
            # after the quad's last exp, ONE batched silu DMA
            def feature_stage(t, x10_tiles, fbqs):
                if t >= NT:
                    return
                q, tl = t // 4, t % 4
                if tl == 0:
                    fbqs[q] = fbqp.tile([128, 2048], bf16, name=f"fbq{q}", tag="fbq")
                fbq = fbqs[q]
                x10_t = x10_tiles[q]
                pt = ptp.tile([128, 512], f32)
                nc.tensor.matmul(
                    pt[:, :],
                    s10_t[0:2 * KF, 0:128],
                    x10_t[0:2 * KF, tl * 512:(tl + 1) * 512],
                    start=True,
                    stop=True,
                )
                nc.scalar.activation(
                    fbq[:, tl * 512:(tl + 1) * 512], pt[:, :], AF.Exp,
                    bias=biasv_t[:, :], scale=1.0,
                )
                if tl == 3:
                    silu_dst = fbq[:, :].rearrange(
                        "(u p) (t c) -> u p t c", u=2, t=4
                    )[:, 12, :, :]
                    silu_src = xsilu[0:1, q * 4096:(q + 1) * 4096].rearrange(
                        "one (t u c) -> (one u) t c", t=4, u=2
                    )
                    nc.scalar.dma_start(out=silu_dst, in_=silu_src)

            x10_tiles = {q: issue_x10_load(q) for q in range(3)}
            wts = {0: issue_wt_load(0), 1: issue_wt_load(1)}
            fbqs = {0: fbq0_t}
            for t0 in range(4, 8):
                feature_stage(t0, x10_tiles, fbqs)

            ncopy = 0
            nact_done = 0
            for t in range(NT):
                if t % 4 == 0:
                    x10_tiles[t // 4 + 3] = issue_x10_load(t // 4 + 3)
                if t % 2 == 0:
                    wts[t // 2 + 2] = issue_wt_load(t // 2 + 2)
                q, tl = t // 4, t % 4
                fbq = fbqs[q]
                if tl == 3:
                    fbqs.pop(q)
                wtA, wtB = wts[t // 2]
                if t % 2 == 1:
                    wts.pop(t // 2)
                iw = t * 4
                st0 = stagep.tile([128, 4 * D], bf16, tag="stage")
                st1 = stagep.tile([128, 4 * D], bf16, tag="stage")
                sts = (st0, st1)
                for j2 in range(2):
                    for c in range(2):
                        po = pop.tile([128, 2 * D], f32)
                        for h in range(2):
                            j = 2 * j2 + h
                            u, cb = j % 2, (j // 2) * 256
                            wt = wtA if u == 0 else wtB
                            nc.tensor.matmul(
                                po[:, h * D:(h + 1) * D],
                                fbq[0:128,
                                    tl * 512 + cb + c * 128:
                                    tl * 512 + cb + c * 128 + 128],
                                wt[0:128,
                                   ((t % 2) * 2 + j // 2) * D:
                                   ((t % 2) * 2 + j // 2 + 1) * D],
                                start=True,
                                stop=True,
                            )
                        dst = sts[c][:, 2 * j2 * D:(2 * j2 + 2) * D]
                        # keep ACT(exp+silu+copies) ~= DVE(copies)
                        use_act = ((ncopy + 1) * _NACT) // 256 > nact_done
                        ncopy += 1
                        if use_act:
                            nact_done += 1
                            nc.scalar.copy(dst, po[:, :])
                        else:
                            nc.vector.tensor_scalar_mul(dst, po[:, :], 1.0)
                        if j2 == 1:
                            # c=1 stores ride SWDGE (latency-tolerant)
                            # except near the end, where gpsimd's slow
                            # descriptor path would become the tail drain
                            eng_dma = (
                                nc.sync if (c == 0 or t >= NT - 4) else nc.gpsimd
                            )
                            eng_dma.dma_start(
                                out=out[c * 128:(c + 1) * 128, iw:iw + 4, :],
                                in_=sts[c][:, :],
                            )
                # staged AFTER the tile's copies: the exp would otherwise
                # sit ahead of them in ACT's strict-FIFO queue and, when
                # the PE lags, head-of-line-block the copies -> po backs
                # up -> PE stalls more -> HAM goes cold (vicious cycle)
                feature_stage(t + 8, x10_tiles, fbqs)

    _split_multi_waits(nc)
    return nc


def _split_multi_waits(nc):
    """Walrus TPB instruction structs accept a single sync wait. Hoist all
    but the last wait of any instruction onto same-engine NOPs inserted
    immediately before it (a wait executes before the instruction either
    way, so this is semantically identical)."""
    import dataclasses
    import concourse.bass as bass
    import concourse.mybir as mybir

    tpl = bass.Bass().sync.nop().ins
    k = 0
    for blk in nc.m.functions[0].blocks:
        out_insts = []
        for inst in blk.instructions:
            si = getattr(inst, "sync_info", None)
            if si is not None and len(si.on_wait) > 1:
                for w in si.on_wait[:-1]:
                    out_insts.append(
                        dataclasses.replace(
                            tpl,
                            name=f"nop-w{k}",
                            engine=inst.engine,
                            sync_info=mybir.SyncInfo(on_wait=[w], on_update=[]),
                        )
                    )
                    k += 1
                inst.sync_info = dataclasses.replace(si, on_wait=si.on_wait[-1:])
            out_insts.append(inst)
        blk.instructions[:] = out_insts


def _hi_lo(v, bf16):
    """Double-bf16 split: v ~= hi + lo with |err| <~ |v| * 2^-17."""
    hi = v.astype(bf16)
    lo = (v - hi.astype(np.float32)).astype(bf16)
    return hi, lo


def _fit_basis(x, grid):
    """Ridge-fit the projection T (M x G) of the grid Gaussians onto M
    coarse sigma-1 Gaussians, weighted by the empirical x distribution."""
    xf = x.ravel().astype(np.float64)
    xs_emp = xf[::17][:30000]
    lo, hi = xf.min() - 0.3, xf.max() + 0.3
    xs_uni = np.linspace(lo, hi, 4000)
    xs = np.concatenate([xs_emp, xs_uni])
    w_s = np.concatenate(
        [np.full(xs_emp.size, 1.0), np.full(xs_uni.size, 0.05)]
    )
    g64 = grid.astype(np.float64)
    ctr = np.linspace(g64.min() - 0.5, g64.max() + 0.5, M)
    PHI = np.exp(-0.5 * (xs[:, None] - g64[None, :]) ** 2)
    PSI = np.exp(-0.5 * (xs[:, None] - ctr[None, :]) ** 2)
    Aw = PSI * np.sqrt(w_s)[:, None]
    Bw = PHI * np.sqrt(w_s)[:, None]
    T = np.linalg.solve(Aw.T @ Aw + 1e-4 * np.eye(M), Aw.T @ Bw)
    return ctr, T


def _prep_inputs(x, base_weight, gp_weight, grid):
    import ml_dtypes

    bf16 = ml_dtypes.bfloat16
    x = np.ascontiguousarray(np.asarray(x, np.float32))
    base_weight = np.asarray(base_weight, np.float32)
    gp_weight = np.asarray(gp_weight, np.float32)
    grid = np.asarray(grid, np.float32)

    ctr, T = _fit_basis(x, grid)

    # x6 feature order: per tile of 4, strip u holds (f+u, f+u+2)
    permx = np.empty(NF, np.int64)
    k = 0
    for t in range(NT):
        for u in range(2):
            permx[k] = t * 4 + u
            permx[k + 1] = t * 4 + u + 2
            k += 2

    # wcatT feature order: per group of 8 (2 tiles), band A holds
    # [f, f+2, f+4, f+6], band B [f+1, f+3, f+5, f+7], each contiguous
    permw = np.empty(NF, np.int64)
    k = 0
    for g in range(NT // 2):
        f = g * 8
        for u in range(2):
            for v in (0, 2, 4, 6):
                permw[k] = f + u + v
                k += 1

    # c[m, i, d] = sum_g T[m, g] gp_w[i, g, d]; row M = base_weight
    cw = np.einsum("mg,igd->mid", T, gp_weight.astype(np.float64))
    wcat = np.concatenate([cw, base_weight[None].astype(np.float64)], axis=0)
    wcatT = np.ascontiguousarray(
        wcat[:, permw, :].reshape(KW, NF * D).astype(bf16)
    )

    h_hi, h_lo = _hi_lo(ctr.astype(np.float32), bf16)
    # double-height stationary (10, 128): rows 0..4 pair with strip-0
    # moving rows and activate cols 0..M-1 (PSUM partitions 0..M-1);
    # rows 5..9 pair with strip-1 moving rows, cols 64..64+M-1
    s10 = np.zeros((2 * KF, 128), bf16)
    for u in range(2):
        o = 64 * u
        r = KF * u
        s10[r + 0, o:o + M] = np.ones(M, bf16)
        s10[r + 1, o:o + M] = np.ones(M, bf16)
        s10[r + 2, o:o + M] = h_hi
        s10[r + 3, o:o + M] = h_lo
        s10[r + 4, o:o + M] = h_hi
    s10 = np.ascontiguousarray(s10)

    biasv = np.full((128, 1), -30.0, np.float32)
    for u in range(2):
        biasv[64 * u:64 * u + M, 0] = (-0.5 * ctr * ctr).astype(np.float32)
    biasv = np.ascontiguousarray(biasv)

    # even/odd feature splits per tile: strip 0 moving = [f, f+2],
    # strip 1 moving = [f+1, f+3]
    perm_ev = np.empty(NF // 2, np.int64)
    perm_od = np.empty(NF // 2, np.int64)
    k = 0
    for t in range(NT):
        perm_ev[k] = t * 4
        perm_ev[k + 1] = t * 4 + 2
        perm_od[k] = t * 4 + 1
        perm_od[k + 1] = t * 4 + 3
        k += 2

    def _x5_rows(xTs):
        mx2 = (-0.5 * xTs * xTs).ravel()
        xr = xTs.ravel()
        mx2_hi, mx2_lo = _hi_lo(mx2, bf16)
        x_hi, x_lo = _hi_lo(xr, bf16)
        return [mx2_hi, mx2_lo, x_hi, x_hi, x_lo]

    in_maps = []
    for cidx in range(NCORES):
        xT = np.ascontiguousarray(x[cidx * BL:(cidx + 1) * BL, :].T)  # (NF, BL)
        xTp = xT[permx]
        x10 = np.ascontiguousarray(
            np.stack(_x5_rows(xT[perm_ev]) + _x5_rows(xT[perm_od]))
        )
        xsilu = np.ascontiguousarray(
            (xTp / (1.0 + np.exp(-xTp))).ravel().astype(bf16).reshape(1, -1)
        )
        # host-precomputed quad-0 feature tile (mirrors the device math:
        # bf16 operands, f32 args, exp, bf16 store, silu rows overwritten)
        x10f = x10.astype(np.float32)
        fbq0 = np.empty((128, 2048), bf16)
        for tl in range(4):
            pt = s10.astype(np.float32).T @ x10f[:, tl * 512:(tl + 1) * 512]
            fbq0[:, tl * 512:(tl + 1) * 512] = np.exp(pt + biasv).astype(bf16)
        ssrc = np.asarray(xsilu[0, 0:4096].reshape(4, 2, 512))
        for u in range(2):
            fbq0[64 * u + 12, :] = ssrc[:, u, :].reshape(4 * 512)
        fbq0 = np.ascontiguousarray(fbq0)
        in_maps.append(
            {"x10": x10, "xsilu": xsilu, "wcatT": wcatT, "s10": s10,
             "biasv": biasv, "zpad": np.zeros((32 - KW, 4 * D), bf16),
             "fbq0": fbq0}
        )
    return in_maps


def _run(in_maps, **kw):
    from concourse.bass_utils import run_bass_kernel_spmd

    if "nc" not in _cache:
        _cache["nc"] = _build()
    return run_bass_kernel_spmd(_cache["nc"], in_maps, list(range(NCORES)), **kw)


def kernel(x, base_weight, gp_weight, grid):
    in_maps = _prep_inputs(x, base_weight, gp_weight, grid)
    res = _run(in_maps)
    return np.concatenate(
        [np.asarray(r["out"]).astype(np.float32) for r in res.results], axis=0
    )
